# revision 1
# baseline (speedup 1.0000x reference)
"""BEiT-style windowed attention (B=32, N=577, D=768, 12 heads) on 8 TRN2 cores.

Strategy: pure data-parallel over batch (4 batch elements per core, no
collectives). All matmuls in bf16 (fp32 PSUM accumulate), softmax in fp32.

Per-core dataflow (all activations kept SBUF-resident per batch element):
  qkT  [d,tok] = W_qkv(T) @ xT          (q/k produced transposed)
  v    [tok,d] = xT(T) @ W_qkvT          (v produced natural, +ones col)
  S.T  [k,q]   = kT(T) @ qT              (scores transposed, K=64)
  P    = exp(S.T + relbias.T)            (unsafe softmax: logits are tiny)
  O_un [q,d+1] = P(T) @ v_aug            (ones col gives rowsum per q)
  O    = O_un[:, :64] * recip(rowsum)    (per-partition scale)
  OT   = transpose(O)                    (PE transposes, 30 per batch)
  out  [tok,od] = OT(T) @ W_projT + b    (natural layout -> contiguous DMA)

Host-side prep: transposed/bf16 weights, x transposed to [d,tok] tiles,
relative-position bias table gathered + transposed, softmax scale folded
into the q rows of W_qkv.
"""

import numpy as np
import ml_dtypes

import concourse.bass as bass
import concourse.tile as tile
from concourse import bacc
from concourse import mybir
from concourse.bass_utils import run_bass_kernel_spmd
from concourse.masks import make_identity

B, N, D = 32, 577, 768
NH, DH = 12, 64
NCORES = 8
BL = B // NCORES            # 4 batch elements per core
SCALE = DH ** -0.5
KT = D // 128               # 6 contraction tiles over D
TT = (N + 127) // 128       # 5 token tiles (4x128 + 65)
BF16 = ml_dtypes.bfloat16

F32 = mybir.dt.float32
BF = mybir.dt.bfloat16


def tok_m(t):
    return min(128, N - 128 * t)


def _build_nc():
    nc = bacc.Bacc()

    xT_d = nc.declare_dram_parameter("xT", [BL, 128, KT, N], BF, isOutput=False)
    wqkv_d = nc.declare_dram_parameter("wqkv", [128, KT, 3 * D], BF, isOutput=False)
    wproj_d = nc.declare_dram_parameter("wproj", [128, KT, D], BF, isOutput=False)
    biasT_d = nc.declare_dram_parameter("biasT", [128, NH, TT, N], BF, isOutput=False)
    qkvb_d = nc.declare_dram_parameter("qkvb", [128, 18], F32, isOutput=False)
    vb_d = nc.declare_dram_parameter("vb", [1, D], F32, isOutput=False)
    pb_d = nc.declare_dram_parameter("pb", [1, D], F32, isOutput=False)
    out_d = nc.declare_dram_parameter("out", [BL, N, D], F32, isOutput=True)

    Exp = mybir.ActivationFunctionType.Exp
    QCH = [(0, 512), (512, N - 512)]          # free-dim chunks over 577
    DCH = [(0, 512), (512, D - 512)]          # free-dim chunks over 768

    with tile.TileContext(nc) as tc:
        with (
            tc.tile_pool(name="singles", bufs=1) as singles,
            tc.tile_pool(name="xt", bufs=2) as xt_pool,
            tc.tile_pool(name="qkt", bufs=2) as qkt_pool,
            tc.tile_pool(name="vbuf", bufs=1) as v_pool,
            tc.tile_pool(name="exps", bufs=10) as exps_pool,
            tc.tile_pool(name="praw", bufs=2) as praw_pool,
            tc.tile_pool(name="obuf", bufs=1) as o_pool,
            tc.tile_pool(name="otb", bufs=1) as ot_pool,
            tc.tile_pool(name="outs", bufs=2) as out_pool,
            tc.tile_pool(name="small", bufs=4) as small_pool,
            tc.tile_pool(name="ps512", bufs=2, space="PSUM") as ps512,
            tc.tile_pool(name="ps128", bufs=2, space="PSUM") as ps128,
            tc.tile_pool(name="psS", bufs=2, space="PSUM") as psS_pool,
        ):
            # ---- one-time loads (small first; big biasT on the second
            # HWDGE queue so it streams while qkv starts) ----
            qkvb = singles.tile([128, 18], F32)
            nc.sync.dma_start(out=qkvb, in_=qkvb_d[:])
            vbias = singles.tile([128, D], F32)
            nc.sync.dma_start(out=vbias, in_=vb_d[:].to_broadcast([128, D]))
            pbias = singles.tile([128, D], F32)
            nc.sync.dma_start(out=pbias, in_=pb_d[:].to_broadcast([128, D]))
            w_qkv = singles.tile([128, KT, 3 * D], BF)
            nc.sync.dma_start(out=w_qkv, in_=wqkv_d[:])
            w_proj = singles.tile([128, KT, D], BF)
            nc.sync.dma_start(out=w_proj, in_=wproj_d[:])
            biasT = singles.tile([128, NH, TT, N], BF)
            nc.scalar.dma_start(out=biasT, in_=biasT_d[:])
            ident = singles.tile([128, 128], BF)
            make_identity(nc, ident)

            for b in range(BL):
                xT = xt_pool.tile([128, KT, N], BF)
                nc.sync.dma_start(out=xT, in_=xT_d[b])

                # ---- qkv projection: q,k transposed [d, tok] ----
                qkT = qkt_pool.tile([128, 2 * KT, N], BF)
                for mt in range(2 * KT):
                    pss = [ps512.tile([128, 512], F32, name="ps_qk0", tag="a"),
                           ps128.tile([128, 128], F32, name="ps_qk1", tag="b")]
                    for kt in range(KT):
                        for ci, (c0, w) in enumerate(QCH):
                            nc.tensor.matmul(
                                pss[ci][:, :w],
                                w_qkv[:, kt, 128 * mt:128 * (mt + 1)],
                                xT[:, kt, c0:c0 + w],
                                start=(kt == 0), stop=(kt == KT - 1),
                            )
                    for ci, (c0, w) in enumerate(QCH):
                        nc.vector.tensor_add(
                            qkT[:, mt, c0:c0 + w], pss[ci][:, :w],
                            qkvb[:, mt:mt + 1].to_broadcast([128, w]),
                        )

                # ---- qkv projection: v natural [tok, d], strided + ones col ----
                v_sb = v_pool.tile([128, TT, NH * 65], BF)
                v_str = v_sb.rearrange("p t (h c) -> p t h c", c=65)
                nc.vector.memset(v_str[:, :, :, 64:65], 1.0)
                for tt in range(TT):
                    m = tok_m(tt)
                    pss = [ps512.tile([128, 512], F32, name="ps_v0", tag="a"),
                           ps512.tile([128, 512], F32, name="ps_v1", tag="a")]
                    for kt in range(KT):
                        for ci, (c0, w) in enumerate(DCH):
                            nc.tensor.matmul(
                                pss[ci][:m, :w],
                                xT[:, kt, 128 * tt:128 * tt + m],
                                w_qkv[:, kt, 2 * D + c0:2 * D + c0 + w],
                                start=(kt == 0), stop=(kt == KT - 1),
                            )
                    for ci, (c0, w) in enumerate(DCH):
                        nh0, nh1 = c0 // 64, (c0 + w) // 64
                        nc.vector.tensor_add(
                            v_str[:m, tt, nh0:nh1, 0:64],
                            pss[ci][:m, :w].rearrange("p (h c) -> p h c", c=64),
                            vbias[:m, c0:c0 + w].rearrange("p (h c) -> p h c", c=64),
                        )

                # ---- attention per head ----
                o_sb = o_pool.tile([128, TT, D], BF)
                for h in range(NH):
                    qT = qkT[64 * (h % 2):64 * (h % 2) + 64, h // 2, :]
                    kTh = qkT[64 * (h % 2):64 * (h % 2) + 64, KT + h // 2, :]
                    expS = [exps_pool.tile([128, N], BF, name="expS", tag="es")
                            for _ in range(TT)]
                    for kt in range(TT):
                        km = tok_m(kt)
                        ps_s = psS_pool.tile([128, N], F32, name="ps_s")
                        for ci, (c0, w) in enumerate(QCH):
                            nc.tensor.matmul(
                                ps_s[:km, c0:c0 + w],
                                kTh[:, 128 * kt:128 * kt + km],
                                qT[:, c0:c0 + w],
                                start=True, stop=True,
                            )
                        praw = praw_pool.tile([128, N], BF)
                        nc.scalar.activation(praw[:km, :], ps_s[:km, :], Exp)
                        # multiply in exp(rel_bias), precomputed on host;
                        # spread across DVE and the otherwise-idle GpSimd
                        eng = nc.gpsimd if kt % 3 == 2 else nc.vector
                        eng.tensor_mul(
                            expS[kt][:km, :], praw[:km, :],
                            biasT[:km, h, kt, :],
                        )
                    # P @ V_aug  (out natural [q, 64+1]; col 64 = rowsum)
                    for qt in range(TT):
                        qm = tok_m(qt)
                        ps_o = ps128.tile([128, 128], F32, name="ps_o", tag="b")
                        for kt in range(TT):
                            km = tok_m(kt)
                            nc.tensor.matmul(
                                ps_o[:qm, :65],
                                expS[kt][:km, 128 * qt:128 * qt + qm],
                                v_sb[:km, kt, 65 * h:65 * h + 65],
                                start=(kt == 0), stop=(kt == TT - 1),
                            )
                        rcp = small_pool.tile([128, 1], F32)
                        nc.vector.reciprocal(rcp[:qm], ps_o[:qm, 64:65])
                        nc.vector.tensor_mul(
                            o_sb[:qm, qt, 64 * h:64 * h + 64],
                            ps_o[:qm, 0:64],
                            rcp[:qm, 0:1].to_broadcast([qm, 64]),
                        )

                # ---- transpose O -> OT [d, tok] ----
                oT = ot_pool.tile([128, KT, N], BF)
                for qt in range(TT):
                    qm = tok_m(qt)
                    for dt in range(KT):
                        ps_t = ps128.tile([128, 128], BF, name="ps_t", tag="b")
                        nc.tensor.transpose(
                            ps_t[:, :qm],
                            o_sb[:qm, qt, 128 * dt:128 * (dt + 1)],
                            ident[:qm, :qm],
                        )
                        nc.scalar.activation(
                            oT[:, dt, 128 * qt:128 * qt + qm], ps_t[:, :qm],
                            mybir.ActivationFunctionType.Copy,
                        )

                # ---- output projection ----
                for tt in range(TT):
                    m = tok_m(tt)
                    out_sb = out_pool.tile([128, D], F32)
                    pss = [ps512.tile([128, 512], F32, name="ps_p0", tag="a"),
                           ps512.tile([128, 512], F32, name="ps_p1", tag="a")]
                    for kt in range(KT):
                        for ci, (c0, w) in enumerate(DCH):
                            nc.tensor.matmul(
                                pss[ci][:m, :w],
                                oT[:, kt, 128 * tt:128 * tt + m],
                                w_proj[:, kt, c0:c0 + w],
                                start=(kt == 0), stop=(kt == KT - 1),
                            )
                    for ci, (c0, w) in enumerate(DCH):
                        nc.vector.tensor_add(
                            out_sb[:m, c0:c0 + w], pss[ci][:m, :w],
                            pbias[:m, c0:c0 + w],
                        )
                    nc.sync.dma_start(
                        out=out_d[b, 128 * tt:128 * tt + m, :],
                        in_=out_sb[:m, :],
                    )
    nc.finalize()
    return nc


_NC_CACHE = {}


def _get_nc():
    if "nc" not in _NC_CACHE:
        _NC_CACHE["nc"] = _build_nc()
    return _NC_CACHE["nc"]


def _prep_shared(qkv_w, q_bias, v_bias, rpb_table, proj_w, proj_b, rel_index):
    qkv_w = np.asarray(qkv_w, dtype=np.float32).copy()
    qkv_w[:D] *= SCALE                      # fold softmax scale into q rows
    qkv_bias = np.concatenate([
        np.asarray(q_bias, np.float32) * SCALE,
        np.zeros(D, np.float32),
        np.asarray(v_bias, np.float32),
    ])
    # [128, KT, 3D]: w[p, kt, m] = qkv_w[m, kt*128+p]
    wqkv = np.ascontiguousarray(
        qkv_w.T.reshape(KT, 128, 3 * D).transpose(1, 0, 2)).astype(BF16)
    wproj = np.ascontiguousarray(
        np.asarray(proj_w, np.float32).T.reshape(KT, 128, D)
        .transpose(1, 0, 2)).astype(BF16)
    qkvb = np.ascontiguousarray(qkv_bias.reshape(18, 128).T).astype(np.float32)
    # relative position bias, transposed to [k, q] and padded to 640 rows
    rb = np.asarray(rpb_table, np.float32)[
        np.asarray(rel_index).reshape(-1)].reshape(N, N, NH)  # [q, k, h]
    rbp = np.zeros((TT * 128, N, NH), np.float32)
    rbp[:N] = rb.transpose(1, 0, 2)                            # [k, q, h]
    biasT = np.ascontiguousarray(
        np.exp(rbp.reshape(TT, 128, N, NH).transpose(1, 3, 0, 2))).astype(BF16)
    vb = np.ascontiguousarray(qkv_bias[2 * D:].reshape(1, D)).astype(np.float32)
    pb = np.ascontiguousarray(np.asarray(proj_b, np.float32).reshape(1, D))
    return wqkv, wproj, qkvb, biasT, vb, pb


def _make_in_maps(inputs):
    x = np.asarray(inputs["x"], dtype=np.float32)
    wqkv, wproj, qkvb, biasT, vb, pb = _prep_shared(
        inputs["qkv_w"], inputs["q_bias"], inputs["v_bias"],
        inputs["rpb_table"], inputs["proj_w"], inputs["proj_b"],
        inputs["rel_index"])

    in_maps = []
    for i in range(NCORES):
        xs = x[i * BL:(i + 1) * BL]                            # [BL, N, D]
        xT = np.ascontiguousarray(
            xs.transpose(0, 2, 1).reshape(BL, KT, 128, N)
            .transpose(0, 2, 1, 3)).astype(BF16)               # [BL,128,KT,N]
        in_maps.append({
            "xT": xT, "wqkv": wqkv, "wproj": wproj, "biasT": biasT,
            "qkvb": qkvb, "vb": vb, "pb": pb,
        })

    return in_maps


def kernel(**inputs):
    in_maps = _make_in_maps(inputs)
    nc = _get_nc()
    res = run_bass_kernel_spmd(nc, in_maps, core_ids=list(range(NCORES)))
    out = np.concatenate([res.results[i]["out"] for i in range(NCORES)], axis=0)
    return np.ascontiguousarray(out.astype(np.float32))


def kernel_traced(**inputs):
    """Like kernel() but also returns (out, BassKernelResults with profile)."""
    in_maps = _make_in_maps(inputs)
    nc = _get_nc()
    res = run_bass_kernel_spmd(nc, in_maps, core_ids=list(range(NCORES)),
                               trace=True)
    out = np.concatenate([res.results[i]["out"] for i in range(NCORES)], axis=0)
    return np.ascontiguousarray(out.astype(np.float32)), res



# revision 15
# speedup vs baseline: 1.2580x; 1.2580x over previous
"""BEiT-style windowed attention (B=32, N=577, D=768, 12 heads) on 8 TRN2 cores.

Strategy: pure data-parallel over batch (4 batch elements per core, no
collectives). qkv projection in fp8e4 DoubleRow matmuls with 3-term residual
compensation (hi@hi + lo@hi + hi@lo) for bf16-level accuracy at 1.33x bf16
speed; scores / P@V / out-proj in bf16 (fp8 there fails the 2e-2 gate:
logit-error rms transfers ~1:1 into the output metric). Softmax is unsafe-exp
with the scale constants folded into the fp8 weight encodings
(q: SQ*softmax_scale, k: SK, v: SV) and unfolded via the exp() scale immediate
and the final evacuation.

Per-core dataflow per batch element:
  qkT [d,tok]  = 9 fp8-DR matmuls per 128-col block (x8/xr8 vs W8hi/W8lo)
  v   [tok,d]  = same, transposed roles; kept at SV*v (ones col appended)
  S.T [k,q]    = kT(T) @ qT in bf16 (K=64)
  praw         = exp(S.T * 1/(SQ*SK))  on scalar engine
  P            = praw * exp(rel_bias).T   (DVE / GpSimd split)
  O_un [q,65]  = P(T) @ v_aug  (col 64 = rowsum)
  O            = O_un[:, :64] * recip(rowsum)   (= SV * O_norm)
  OT           = PE transpose, evacuated on DVE
  out  [tok,d] = OT(T) @ W_projT * 1/SV  -> bf16 -> HBM

v-bias and proj-bias are folded into a host-side row add (out += v_bias @
proj_w.T + proj_b); q-bias is applied in the qk evacuation (per-partition).

Issue order is software-pipelined: batch b+1's qkv matmul groups are
interleaved into batch b's attention heads so the PE never idles waiting on
exp/mul, and the second half of the bias table is streamed from DRAM (first
half resident) to fit SBUF.
"""

import numpy as np
import ml_dtypes

import concourse.bass as bass
import concourse.tile as tile
from concourse import bacc
from concourse import mybir
from concourse.bass_utils import run_bass_kernel_spmd
from concourse.masks import make_identity

B, N, D = 32, 577, 768
NH, DH = 12, 64
NCORES = 8
BL = B // NCORES            # 4 batch elements per core
SCALE = DH ** -0.5
KT = D // 128                # 6 contraction tiles over D
TT = (N + 127) // 128        # 5 token tiles (4x128 + 65)
NP = 592                     # fp8 x token pad (DoubleRow slice stride % 16)
SQ, SK, SV = 256.0, 32.0, 64.0   # fp8 scale folds for q / k / v
CW = 4608                    # w8 cols: qk-hi 1536 | qk-lo 1536 | v-hi 768 | v-lo 768
NRES = 6                     # bias-table heads resident in SBUF; rest streamed

BF16 = ml_dtypes.bfloat16
F8 = ml_dtypes.float8_e4m3   # TRN e4m3 (max normal 240)

F32 = mybir.dt.float32
BF = mybir.dt.bfloat16
E4 = mybir.dt.float8e4
DR = mybir.MatmulPerfMode.DoubleRow
ADD = mybir.AluOpType.add
MULT = mybir.AluOpType.mult


def tok_m(t):
    return min(128, N - 128 * t)


def _build_nc():
    nc = bacc.Bacc()

    x8_d = nc.declare_dram_parameter("x8", [BL, 128, KT, NP], E4, isOutput=False)
    xr8_d = nc.declare_dram_parameter("xr8", [BL, 128, KT, NP], E4, isOutput=False)
    w8qk_d = nc.declare_dram_parameter("w8qk", [128, KT, 3072], E4, isOutput=False)
    w8v_d = nc.declare_dram_parameter("w8v", [128, KT, 1536], E4, isOutput=False)
    wp_d = nc.declare_dram_parameter("wp", [128, KT, D], BF, isOutput=False)
    qkvb_d = nc.declare_dram_parameter("qkvb", [128, 12], F32, isOutput=False)
    bres_d = nc.declare_dram_parameter("bres", [128, NRES, TT, N], BF, isOutput=False)
    bstr_d = nc.declare_dram_parameter("bstr", [NH - NRES, 128, TT, N], BF,
                                       isOutput=False)
    out_d = nc.declare_dram_parameter("out", [BL, N, D], BF, isOutput=True)

    Exp = mybir.ActivationFunctionType.Exp
    QCH_DR = [(0, 256), (256, 256), (512, N - 512)]   # qk DR out chunks
    VCH_DR = [(0, 256), (256, 256), (512, 256)]       # v DR out chunks
    SCH = [(0, 512), (512, N - 512)]                  # scores bf16 chunks
    DCH = [(0, 512), (512, 256)]                      # proj bf16 chunks

    with tile.TileContext(nc) as tc:
        with (
            tc.tile_pool(name="singles", bufs=1) as singles,
            tc.tile_pool(name="xp", bufs=2) as x_pool,
            tc.tile_pool(name="qktp", bufs=2) as qkt_pool,
            tc.tile_pool(name="vp", bufs=2) as v_pool,
            tc.tile_pool(name="bstrp", bufs=3) as bstr_pool,
            tc.tile_pool(name="prawp", bufs=3) as praw_pool,
            tc.tile_pool(name="expsp", bufs=10) as exps_pool,
            tc.tile_pool(name="op", bufs=2) as o_pool,
            tc.tile_pool(name="otp", bufs=2) as ot_pool,
            tc.tile_pool(name="outp", bufs=2) as out_pool,
            tc.tile_pool(name="smallp", bufs=4) as small_pool,
            tc.tile_pool(name="psA", bufs=3, space="PSUM") as psA,
            tc.tile_pool(name="psB", bufs=2, space="PSUM") as psB,
        ):
            # ---- one-time loads (ordered so batch-0 qk work starts ASAP:
            # x first, then qk weight cols; v cols / bias / proj stream in
            # behind while the first qk matmul groups run) ----
            x0 = None
            w8qk = singles.tile([128, KT, 3072], E4)
            w8v = singles.tile([128, KT, 1536], E4)
            qkvb = singles.tile([128, 12], F32)
            wproj = singles.tile([128, KT, D], BF)
            bres = singles.tile([128, NRES, TT, N], BF)
            ident = singles.tile([128, 128], BF)

            xs = {}      # b -> (x8 tile, xr8 tile)
            qkts = {}    # b -> qkT tile [128, 12, N] bf16
            vs = {}      # b -> v_sb tile (strided view with ones col)
            os_ = {}     # b -> o_sb tile
            bias_tiles = {}   # (b, h) -> streamed bias tile

            def load_x(b):
                x8 = x_pool.tile([128, KT, NP], E4, name="x8t", tag="x8")
                xr8 = x_pool.tile([128, KT, NP], E4, name="xr8t", tag="xr8")
                nc.sync.dma_start(out=x8, in_=x8_d[b])
                nc.sync.dma_start(out=xr8, in_=xr8_d[b])
                xs[b] = (x8, xr8)

            def prefetch_bias(b, h):
                t = bstr_pool.tile([128, TT, N], BF, name="biash")
                nc.sync.dma_start(out=t, in_=bstr_d[h - NRES])
                bias_tiles[(b, h)] = t

            def bias_ap(b, h):
                if h < NRES:
                    return bres[:, h]
                return bias_tiles[(b, h)]

            def emit_qk_group(b, mt):
                """One 128-col block of the q/k projection: 27 fp8-DR matmuls."""
                x8, xr8 = xs[b]
                if b not in qkts:
                    qkts[b] = qkt_pool.tile([128, 2 * KT, N], BF, name="qkT")
                ps = psA.tile([128, N], F32, name="ps_qk", tag="A")
                for c0, w in QCH_DR:
                    idx = 0
                    for coff, xt in ((0, x8), (1536, x8), (0, xr8)):
                        for kp in range(3):
                            nc.tensor.matmul(
                                ps[:, c0:c0 + w],
                                w8qk[:, 2 * kp:2 * kp + 2,
                                     coff + 128 * mt:coff + 128 * mt + 128],
                                xt[:, 2 * kp:2 * kp + 2, c0:c0 + w],
                                start=(idx == 0), stop=(idx == 8),
                                perf_mode=DR,
                            )
                            idx += 1
                nc.vector.tensor_scalar(
                    out=qkts[b][:, mt, :], in0=ps[:, :N],
                    scalar1=qkvb[:, mt:mt + 1], scalar2=None, op0=ADD,
                )

            def emit_v_group(b, tt):
                """One 128-token block of the v projection (kept at SV*v)."""
                x8, xr8 = xs[b]
                if b not in vs:
                    v_sb = v_pool.tile([128, TT, NH * 65], BF, name="v_sb")
                    v_str = v_sb.rearrange("p t (h c) -> p t h c", c=65)
                    nc.vector.memset(v_str[:, :, :, 64:65], 1.0)
                    vs[b] = v_str
                v_str = vs[b]
                m = tok_m(tt)
                ps = psA.tile([128, D], F32, name="ps_v", tag="A")
                for c0, w in VCH_DR:
                    idx = 0
                    for xt, coff in ((x8, 0), (xr8, 0), (x8, 768)):
                        for kp in range(3):
                            nc.tensor.matmul(
                                ps[:m, c0:c0 + w],
                                xt[:, 2 * kp:2 * kp + 2, 128 * tt:128 * tt + m],
                                w8v[:, 2 * kp:2 * kp + 2, coff + c0:coff + c0 + w],
                                start=(idx == 0), stop=(idx == 8),
                                perf_mode=DR,
                            )
                            idx += 1
                nc.vector.tensor_scalar(
                    out=v_str[:m, tt, :, 0:64],
                    in0=ps[:m, :].rearrange("p (h c) -> p h c", c=64),
                    scalar1=1.0, scalar2=None, op0=MULT,
                )

            def emit_scores(b, h):
                """S.T tiles -> exp -> bias-mul for one head; returns expS tiles."""
                qkT = qkts[b]
                po = 64 * (h % 2)
                qT = qkT[po:po + 64, h // 2, :]
                kTh = qkT[po:po + 64, KT + h // 2, :]
                bh = bias_ap(b, h)
                expS = [exps_pool.tile([128, N], BF, name="expS", tag="es")
                        for _ in range(TT)]
                for kt in range(TT):
                    km = tok_m(kt)
                    ps_s = psA.tile([128, N], F32, name="ps_s", tag="A")
                    for c0, w in SCH:
                        nc.tensor.matmul(
                            ps_s[:km, c0:c0 + w],
                            kTh[:, 128 * kt:128 * kt + km],
                            qT[:, c0:c0 + w],
                            start=True, stop=True,
                        )
                    praw = praw_pool.tile([128, N], BF)
                    nc.scalar.activation(praw[:km, :], ps_s[:km, :], Exp,
                                         scale=1.0 / (SQ * SK))
                    eng = nc.vector if kt == 3 else nc.gpsimd
                    eng.tensor_mul(expS[kt][:km, :], praw[:km, :], bh[:km, kt, :])
                return expS

            def emit_pv(b, h, expS):
                if b not in os_:
                    os_[b] = o_pool.tile([128, TT, D], BF, name="o_sb")
                o_sb = os_[b]
                v_str = vs[b]
                for qt in range(TT):
                    qm = tok_m(qt)
                    ps_o = psB.tile([128, 128], F32, name="ps_o", tag="B")
                    for kt in range(TT):
                        km = tok_m(kt)
                        nc.tensor.matmul(
                            ps_o[:qm, :65],
                            expS[kt][:km, 128 * qt:128 * qt + qm],
                            v_str[:km, kt, h, :],
                            start=(kt == 0), stop=(kt == TT - 1),
                        )
                    rcp = small_pool.tile([128, 1], F32)
                    nc.vector.reciprocal(rcp[:qm], ps_o[:qm, 64:65])
                    nc.vector.tensor_mul(
                        o_sb[:qm, qt, 64 * h:64 * h + 64],
                        ps_o[:qm, 0:64],
                        rcp[:qm, 0:1].to_broadcast([qm, 64]),
                    )

            def emit_transpose(b, qt):
                oT = ots[b]
                qm = tok_m(qt)
                o_sb = os_[b]
                for dt_ in range(KT):
                    ps_t = psB.tile([128, 128], BF, name="ps_t", tag="B")
                    nc.tensor.transpose(
                        ps_t[:, :qm],
                        o_sb[:qm, qt, 128 * dt_:128 * (dt_ + 1)],
                        ident[:qm, :qm],
                    )
                    if dt_ % 2:
                        nc.scalar.copy(oT[:, dt_, 128 * qt:128 * qt + qm],
                                       ps_t[:, :qm])
                    else:
                        nc.vector.tensor_scalar(
                            out=oT[:, dt_, 128 * qt:128 * qt + qm],
                            in0=ps_t[:, :qm], scalar1=1.0, scalar2=None,
                            op0=MULT,
                        )

            def emit_proj(b, tt):
                oT = ots[b]
                m = tok_m(tt)
                ps = psA.tile([128, D], F32, name="ps_p", tag="A")
                for kt in range(KT):
                    for c0, w in DCH:
                        nc.tensor.matmul(
                            ps[:m, c0:c0 + w],
                            oT[:, kt, 128 * tt:128 * tt + m],
                            wproj[:, kt, c0:c0 + w],
                            start=(kt == 0), stop=(kt == KT - 1),
                        )
                out_sb = out_pool.tile([128, D], BF)
                nc.vector.tensor_scalar(
                    out=out_sb[:m, :], in0=ps[:m, :],
                    scalar1=1.0 / SV, scalar2=None, op0=MULT,
                )
                nc.sync.dma_start(out=out_d[b, 128 * tt:128 * tt + m, :],
                                  in_=out_sb[:m, :])

            # ---- software-pipelined emission ----
            # attn(b) is interleaved with qkv(b+1) and transpose+proj(b-1) so
            # the PE stream has fill work while exp/mul latencies drain.
            load_x(0)
            nc.sync.dma_start(out=w8qk, in_=w8qk_d[:])
            nc.sync.dma_start(out=qkvb, in_=qkvb_d[:])
            nc.sync.dma_start(out=w8v, in_=w8v_d[:])
            nc.sync.dma_start(out=bres, in_=bres_d[:])
            nc.sync.dma_start(out=wproj, in_=wp_d[:])
            make_identity(nc, ident)

            ots = {}

            def tp_items(b):
                ots[b] = ot_pool.tile([128, KT, N], BF, name="oT",
                                      uniquify=True)
                items = []
                for qt in range(TT):
                    items.append((emit_transpose, (b, qt)))
                    items.append((emit_proj, (b, qt)))
                return items

            for mt in range(2 * KT):
                emit_qk_group(0, mt)
            for tt in range(TT):
                emit_v_group(0, tt)

            for b in range(BL):
                nxt = []
                if b >= 1:
                    nxt += tp_items(b - 1)
                if b + 1 < BL:
                    load_x(b + 1)
                    nxt += [(emit_qk_group, (b + 1, mt)) for mt in range(2 * KT)]
                    nxt += [(emit_v_group, (b + 1, tt)) for tt in range(TT)]
                for hh in range(NRES, min(NRES + 3, NH)):
                    prefetch_bias(b, hh)
                ni = 0
                for h in range(NH):
                    if NRES + 3 <= h + 3 < NH:
                        prefetch_bias(b, h + 3)
                    expS = emit_scores(b, h)
                    want = (len(nxt) * (h + 1)) // NH
                    while ni < want:
                        f, a = nxt[ni]
                        f(*a)
                        ni += 1
                    emit_pv(b, h, expS)
                while ni < len(nxt):
                    f, a = nxt[ni]
                    f(*a)
                    ni += 1
            for f, a in tp_items(BL - 1):
                f(*a)
    nc.finalize()
    return nc


_NC_CACHE = {}


def _get_nc():
    if "nc" not in _NC_CACHE:
        _NC_CACHE["nc"] = _build_nc()
    return _NC_CACHE["nc"]


def _q8(a):
    return np.asarray(a, np.float32).astype(F8)


def _prep_shared(qkv_w, q_bias, rpb_table, proj_w, rel_index):
    qkv_w = np.asarray(qkv_w, dtype=np.float32)
    Wq = qkv_w[:D].T * (SCALE * SQ)          # [768, 768] in-dim major
    Wk = qkv_w[D:2 * D].T * SK
    Wv = qkv_w[2 * D:].T * SV
    qk = np.concatenate([Wq, Wk], axis=1)     # [768, 1536]
    qk_hi = _q8(qk)
    qk_lo = _q8(qk - qk_hi.astype(np.float32))
    v_hi = _q8(Wv)
    v_lo = _q8(Wv - v_hi.astype(np.float32))
    w8qk = np.ascontiguousarray(
        np.concatenate([qk_hi, qk_lo], axis=1)
        .reshape(KT, 128, 3072).transpose(1, 0, 2))        # [128, KT, 3072]
    w8v = np.ascontiguousarray(
        np.concatenate([v_hi, v_lo], axis=1)
        .reshape(KT, 128, 1536).transpose(1, 0, 2))        # [128, KT, 1536]

    wp = np.ascontiguousarray(
        np.asarray(proj_w, np.float32).T.reshape(KT, 128, D)
        .transpose(1, 0, 2)).astype(BF16)

    qb = np.zeros((128, 12), np.float32)
    qb_scaled = (np.asarray(q_bias, np.float32) * (SCALE * SQ)).reshape(KT, 128)
    qb[:, :KT] = qb_scaled.T

    rb = np.asarray(rpb_table, np.float32)[
        np.asarray(rel_index).reshape(-1)].reshape(N, N, NH)   # [q, k, h]
    M = np.exp(rb).transpose(2, 1, 0)                          # [h, k, q]
    Mp = np.zeros((NH, TT * 128, N), np.float32)
    Mp[:, :N] = M
    arr = Mp.reshape(NH, TT, 128, N).transpose(0, 2, 1, 3).astype(BF16)
    bres = np.ascontiguousarray(arr[:NRES].transpose(1, 0, 2, 3))
    bstr = np.ascontiguousarray(arr[NRES:])
    return w8qk, w8v, wp, qb, bres, bstr


def _make_in_maps(inputs):
    x = np.asarray(inputs["x"], dtype=np.float32)
    w8qk, w8v, wp, qb, bres, bstr = _prep_shared(
        inputs["qkv_w"], inputs["q_bias"], inputs["rpb_table"],
        inputs["proj_w"], inputs["rel_index"])

    in_maps = []
    for i in range(NCORES):
        xsl = x[i * BL:(i + 1) * BL]                    # [BL, N, D]
        xT = np.zeros((BL, D, NP), np.float32)
        xT[:, :, :N] = xsl.transpose(0, 2, 1)
        x8 = xT.astype(F8)
        xr8 = (xT - x8.astype(np.float32)).astype(F8)
        x8 = np.ascontiguousarray(
            x8.reshape(BL, KT, 128, NP).transpose(0, 2, 1, 3))
        xr8 = np.ascontiguousarray(
            xr8.reshape(BL, KT, 128, NP).transpose(0, 2, 1, 3))
        in_maps.append({
            "x8": x8, "xr8": xr8, "w8qk": w8qk, "w8v": w8v, "wp": wp,
            "qkvb": qb, "bres": bres, "bstr": bstr,
        })
    return in_maps


def _finish(inputs, res):
    out = np.concatenate(
        [np.asarray(res.results[i]["out"], np.float32) for i in range(NCORES)],
        axis=0)
    row = (np.asarray(inputs["v_bias"], np.float32)
           @ np.asarray(inputs["proj_w"], np.float32).T
           + np.asarray(inputs["proj_b"], np.float32))
    out += row[None, None, :]
    return np.ascontiguousarray(out)


def kernel(**inputs):
    in_maps = _make_in_maps(inputs)
    nc = _get_nc()
    res = run_bass_kernel_spmd(nc, in_maps, core_ids=list(range(NCORES)))
    return _finish(inputs, res)


def kernel_traced(**inputs):
    """Like kernel() but also returns (out, BassKernelResults with profile)."""
    in_maps = _make_in_maps(inputs)
    nc = _get_nc()
    res = run_bass_kernel_spmd(nc, in_maps, core_ids=list(range(NCORES)),
                               trace=True)
    return _finish(inputs, res), res


# revision 41
# speedup vs baseline: 1.3182x; 1.0479x over previous
"""BEiT-style windowed attention (B=32, N=577, D=768, 12 heads) on 8 TRN2 cores.

Strategy: pure data-parallel over batch (4 batch elements per core, no
collectives). qkv projection in fp8e4 DoubleRow matmuls with 3-term residual
compensation (hi@hi + lo@hi + hi@lo) for bf16-level accuracy at 1.33x bf16
speed; scores / P@V / out-proj in bf16 (fp8 there fails the 2e-2 gate:
logit-error rms transfers ~1:1 into the output metric). Softmax is unsafe-exp
with the scale constants folded into the fp8 weight encodings
(q: SQ*softmax_scale, k: SK, v: SV) and unfolded via the exp() scale immediate
and the final evacuation.

Per-core dataflow per batch element:
  qkT [d,tok]  = 9 fp8-DR matmuls per 128-col block (x8/xr8 vs W8hi/W8lo)
  v   [tok,d]  = same, transposed roles; kept at SV*v (ones col appended)
  S.T [k,q]    = kT(T) @ qT in bf16 (K=64)
  praw         = exp(S.T * 1/(SQ*SK))  on scalar engine
  P            = praw * exp(rel_bias).T   (DVE / GpSimd split)
  O_un [q,65]  = P(T) @ v_aug  (col 64 = rowsum)
  O            = O_un[:, :64] * recip(rowsum)   (= SV * O_norm)
  OT           = PE transpose, evacuated on DVE
  out  [tok,d] = OT(T) @ W_projT * 1/SV  -> bf16 -> HBM

v-bias and proj-bias are folded into a host-side row add (out += v_bias @
proj_w.T + proj_b); q-bias is applied in the qk evacuation (per-partition).

Issue order is software-pipelined: batch b+1's qkv matmul groups are
interleaved into batch b's attention heads so the PE never idles waiting on
exp/mul, and the second half of the bias table is streamed from DRAM (first
half resident) to fit SBUF.
"""

import numpy as np
import ml_dtypes

import concourse.bass as bass
import concourse.tile as tile
from concourse import bacc
from concourse import mybir
from concourse.bass_utils import run_bass_kernel_spmd
from concourse.masks import make_identity

B, N, D = 32, 577, 768
NH, DH = 12, 64
NCORES = 8
BL = B // NCORES            # 4 batch elements per core
SCALE = DH ** -0.5
KT = D // 128                # 6 contraction tiles over D
TT = (N + 127) // 128        # 5 token tiles (4x128 + 65)
NP = 592                     # fp8 x token pad (DoubleRow slice stride % 16)
SQ, SK, SV = 256.0, 32.0, 64.0   # fp8 scale folds for q / k / v
CW = 4608                    # w8 cols: qk-hi 1536 | qk-lo 1536 | v-hi 768 | v-lo 768
NRES = 6                     # bias-table heads resident in SBUF; rest streamed

BF16 = ml_dtypes.bfloat16
F8 = ml_dtypes.float8_e4m3   # TRN e4m3 (max normal 240)

F32 = mybir.dt.float32
BF = mybir.dt.bfloat16
E4 = mybir.dt.float8e4
DR = mybir.MatmulPerfMode.DoubleRow
ADD = mybir.AluOpType.add
MULT = mybir.AluOpType.mult


def tok_m(t):
    return min(128, N - 128 * t)


def _build_nc():
    nc = bacc.Bacc()

    x8_d = nc.declare_dram_parameter("x8", [BL, 128, KT, NP], E4, isOutput=False)
    xr8_d = nc.declare_dram_parameter("xr8", [BL, 128, KT, NP], E4, isOutput=False)
    w8qk_d = nc.declare_dram_parameter("w8qk", [128, KT, 3072], E4, isOutput=False)
    w8v_d = nc.declare_dram_parameter("w8v", [128, KT, 1536], E4, isOutput=False)
    wp_d = nc.declare_dram_parameter("wp", [128, KT, D], BF, isOutput=False)
    qkvb_d = nc.declare_dram_parameter("qkvb", [128, 12], F32, isOutput=False)
    bres_d = nc.declare_dram_parameter("bres", [128, NRES, TT, N], BF, isOutput=False)
    bstr_d = nc.declare_dram_parameter("bstr", [NH - NRES, 128, TT, N], BF,
                                       isOutput=False)
    out_d = nc.declare_dram_parameter("out", [BL, N, D], BF, isOutput=True)

    Exp = mybir.ActivationFunctionType.Exp
    QCH_DR = [(0, 256), (256, 256), (512, N - 512)]   # qk DR out chunks
    VCH_DR = [(0, 256), (256, 256), (512, 256)]       # v DR out chunks
    SCH = [(0, 512), (512, N - 512)]                  # scores bf16 chunks
    DCH = [(0, 512), (512, 256)]                      # proj bf16 chunks

    with tile.TileContext(nc) as tc:
        with (
            tc.tile_pool(name="singles", bufs=1) as singles,
            tc.tile_pool(name="xp", bufs=2) as x_pool,
            tc.tile_pool(name="qktp", bufs=2) as qkt_pool,
            tc.tile_pool(name="vp", bufs=2) as v_pool,
            tc.tile_pool(name="bstrp", bufs=3) as bstr_pool,
            tc.tile_pool(name="prawp", bufs=4) as praw_pool,
            tc.tile_pool(name="expsp", bufs=15) as exps_pool,
            tc.tile_pool(name="op", bufs=2) as o_pool,
            tc.tile_pool(name="otp", bufs=2) as ot_pool,
            tc.tile_pool(name="outp", bufs=2) as out_pool,
            tc.tile_pool(name="smallp", bufs=4) as small_pool,
            tc.tile_pool(name="psA", bufs=3, space="PSUM") as psA,
            tc.tile_pool(name="psB", bufs=2, space="PSUM") as psB,
        ):
            # ---- one-time loads (ordered so batch-0 qk work starts ASAP:
            # x first, then qk weight cols; v cols / bias / proj stream in
            # behind while the first qk matmul groups run) ----
            x0 = None
            w8qk = singles.tile([128, KT, 3072], E4)
            w8v = singles.tile([128, KT, 1536], E4)
            qkvb = singles.tile([128, 12], F32)
            wproj = singles.tile([128, KT, D], BF)
            bres = singles.tile([128, NRES, TT, N], BF)
            ident = singles.tile([128, 128], BF)

            xs = {}      # b -> (x8 tile, xr8 tile)
            qkts = {}    # b -> qkT tile [128, 12, N] bf16
            vs = {}      # b -> v_sb tile (strided view with ones col)
            os_ = {}     # b -> o_sb tile
            bias_tiles = {}   # (b, h) -> streamed bias tile

            def load_x(b, split=False):
                x8 = x_pool.tile([128, KT, NP], E4, name="x8t", tag="x8")
                xr8 = x_pool.tile([128, KT, NP], E4, name="xr8t", tag="xr8")
                nc.sync.dma_start(out=x8, in_=x8_d[b])
                if not split:
                    nc.sync.dma_start(out=xr8, in_=xr8_d[b])
                xs[b] = (x8, xr8)
                return xr8

            def prefetch_bias(b, h):
                t = bstr_pool.tile([128, TT, N], BF, name="biash")
                nc.sync.dma_start(out=t, in_=bstr_d[h - NRES])
                bias_tiles[(b, h)] = t

            def bias_ap(b, h):
                if h < NRES:
                    return bres[:, h]
                return bias_tiles[(b, h)]

            def emit_qk_group(b, mt):
                """One 128-col block of the q/k projection: 27 fp8-DR matmuls."""
                x8, xr8 = xs[b]
                if b not in qkts:
                    qkts[b] = qkt_pool.tile([128, 2 * KT, N], BF, name="qkT")
                ps = psA.tile([128, N], F32, name="ps_qk", tag="A")
                for c0, w in QCH_DR:
                    idx = 0
                    for coff, xt in ((0, x8), (128, x8), (0, xr8)):
                        for kp in range(3):
                            nc.tensor.matmul(
                                ps[:, c0:c0 + w],
                                w8qk[:, 2 * kp:2 * kp + 2,
                                     coff + 256 * mt:coff + 256 * mt + 128],
                                xt[:, 2 * kp:2 * kp + 2, c0:c0 + w],
                                start=(idx == 0), stop=(idx == 8),
                                perf_mode=DR,
                            )
                            idx += 1
                nc.vector.tensor_scalar(
                    out=qkts[b][:, mt, :], in0=ps[:, :N],
                    scalar1=qkvb[:, mt:mt + 1], scalar2=None, op0=ADD,
                )

            def emit_v_group(b, tt):
                """One 128-token block of the v projection (kept at SV*v)."""
                x8, xr8 = xs[b]
                if b not in vs:
                    v_sb = v_pool.tile([128, TT, NH * 65], BF, name="v_sb")
                    v_str = v_sb.rearrange("p t (h c) -> p t h c", c=65)
                    nc.vector.memset(v_str[:, :, :, 64:65], 1.0)
                    vs[b] = v_str
                v_str = vs[b]
                m = tok_m(tt)
                ps = psA.tile([128, D], F32, name="ps_v", tag="A")
                for c0, w in VCH_DR:
                    idx = 0
                    for xt, coff in ((x8, 0), (xr8, 0), (x8, 768)):
                        for kp in range(3):
                            nc.tensor.matmul(
                                ps[:m, c0:c0 + w],
                                xt[:, 2 * kp:2 * kp + 2, 128 * tt:128 * tt + m],
                                w8v[:, 2 * kp:2 * kp + 2, coff + c0:coff + c0 + w],
                                start=(idx == 0), stop=(idx == 8),
                                perf_mode=DR,
                            )
                            idx += 1
                nc.vector.tensor_scalar(
                    out=v_str[:m, tt, :, 0:64],
                    in0=ps[:m, :].rearrange("p (h c) -> p h c", c=64),
                    scalar1=1.0, scalar2=None, op0=MULT,
                )

            def emit_scores(b, h, fill=None):
                """S.T tiles -> exp -> bias-mul for one head; returns expS tiles.

                `fill(n)` emits up to n deferred PE work items; called between
                score tiles so the PE has queued work while psA slots recycle
                at exp() speed.
                """
                qkT = qkts[b]
                po = 64 * (h % 2)
                qT = qkT[po:po + 64, h // 2, :]
                kTh = qkT[po:po + 64, KT + h // 2, :]
                bh = bias_ap(b, h)
                expS = [exps_pool.tile([128, N], BF, name="expS", tag="es")
                        for _ in range(TT)]
                for kt in range(TT):
                    if fill is not None and kt >= 2:
                        # window where scores wait on exp() draining a psA
                        # slot; only psB-based work (P@V of the previous head,
                        # transposes) can actually run here
                        fill(2 if kt > 2 else 1)
                    km = tok_m(kt)
                    ps_s = psA.tile([128, N], F32, name="ps_s", tag="A")
                    for c0, w in SCH:
                        nc.tensor.matmul(
                            ps_s[:km, c0:c0 + w],
                            kTh[:, 128 * kt:128 * kt + km],
                            qT[:, c0:c0 + w],
                            start=True, stop=True,
                        )
                    praw = praw_pool.tile([128, N], BF)
                    nc.scalar.activation(praw[:km, :], ps_s[:km, :], Exp,
                                         scale=1.0 / (SQ * SK))
                    eng = nc.vector if kt == 3 else nc.gpsimd
                    eng.tensor_mul(expS[kt][:km, :], praw[:km, :], bh[:km, kt, :])
                return expS

            def emit_pv_piece(b, h, expS, qt):
                if b not in os_:
                    os_[b] = o_pool.tile([128, TT, D], BF, name="o_sb")
                o_sb = os_[b]
                v_str = vs[b]
                # kt order starts at TT-1: the wait on the last bias-mul is
                # consolidated at one fillable point instead of stalling each
                # qt chain mid-accumulation
                kt_order = [TT - 1] + list(range(TT - 1))
                qm = tok_m(qt)
                ps_o = psB.tile([128, 128], F32, name="ps_o", tag="B")
                for i, kt in enumerate(kt_order):
                    km = tok_m(kt)
                    nc.tensor.matmul(
                        ps_o[:qm, :65],
                        expS[kt][:km, 128 * qt:128 * qt + qm],
                        v_str[:km, kt, h, :],
                        start=(i == 0), stop=(i == TT - 1),
                    )
                rcp = small_pool.tile([128, 1], F32)
                nc.vector.reciprocal(rcp[:qm], ps_o[:qm, 64:65])
                nc.vector.tensor_mul(
                    o_sb[:qm, qt, 64 * h:64 * h + 64],
                    ps_o[:qm, 0:64],
                    rcp[:qm, 0:1].to_broadcast([qm, 64]),
                )

            def emit_transpose(b, qt, tail=False):
                oT = ots[b]
                qm = tok_m(qt)
                o_sb = os_[b]
                for dt_ in range(KT):
                    ps_t = psB.tile([128, 128], BF, name="ps_t", tag="B")
                    nc.tensor.transpose(
                        ps_t[:, :qm],
                        o_sb[:qm, qt, 128 * dt_:128 * (dt_ + 1)],
                        ident[:qm, :qm],
                    )
                    if tail and dt_ % 2:
                        # final batch: Act is idle, split the psum drain
                        nc.scalar.copy(oT[:, dt_, 128 * qt:128 * qt + qm],
                                       ps_t[:, :qm])
                    else:
                        nc.vector.tensor_scalar(
                            out=oT[:, dt_, 128 * qt:128 * qt + qm],
                            in0=ps_t[:, :qm], scalar1=1.0, scalar2=None,
                            op0=MULT,
                        )

            def emit_proj(b, tt):
                oT = ots[b]
                m = tok_m(tt)
                ps = psA.tile([128, D], F32, name="ps_p", tag="A")
                out_sb = out_pool.tile([128, D], BF)
                # chunk-major so the first chunk's evacuation overlaps the
                # second chunk's matmuls
                for c0, w in DCH:
                    for kt in range(KT):
                        nc.tensor.matmul(
                            ps[:m, c0:c0 + w],
                            oT[:, kt, 128 * tt:128 * tt + m],
                            wproj[:, kt, c0:c0 + w],
                            start=(kt == 0), stop=(kt == KT - 1),
                        )
                    nc.vector.tensor_scalar(
                        out=out_sb[:m, c0:c0 + w], in0=ps[:m, c0:c0 + w],
                        scalar1=1.0 / SV, scalar2=None, op0=MULT,
                    )
                nc.sync.dma_start(out=out_d[b, 128 * tt:128 * tt + m, :],
                                  in_=out_sb[:m, :])

            # ---- software-pipelined emission ----
            # attn(b) is interleaved with qkv(b+1) and transpose+proj(b-1) so
            # the PE stream has fill work while exp/mul latencies drain.
            xr8_0 = load_x(0, split=True)
            nc.sync.dma_start(out=w8qk[:, :, 0:768], in_=w8qk_d[:, :, 0:768])
            nc.sync.dma_start(out=xr8_0, in_=xr8_d[0])
            nc.sync.dma_start(out=qkvb, in_=qkvb_d[:])
            nc.sync.dma_start(out=w8qk[:, :, 768:3072],
                              in_=w8qk_d[:, :, 768:3072])
            nc.sync.dma_start(out=w8v, in_=w8v_d[:])
            nc.sync.dma_start(out=bres, in_=bres_d[:])
            nc.sync.dma_start(out=wproj, in_=wp_d[:])
            make_identity(nc, ident)

            ots = {}

            def tp_items(b, tail=False):
                ots[b] = ot_pool.tile([128, KT, N], BF, name="oT",
                                      uniquify=True)
                items = []
                for qt in range(TT):
                    items.append((emit_transpose, (b, qt, tail)))
                    items.append((emit_proj, (b, qt)))
                return items

            # qk-groups for mts needed only from head 4 on are deferred into
            # the OWNING batch's early attention heads, so every attention
            # phase (including the last batch's) has PE fill work.
            LATE_MTS = [2, KT + 2, 3, KT + 3, 4, KT + 4, 5, KT + 5]
            EARLY_MTS = [0, 1, KT, KT + 1]
            deferred = {}   # b -> list of (emit fn, args) pinned to heads 0..3

            for mt in EARLY_MTS:
                emit_qk_group(0, mt)
            for tt in range(TT):
                emit_v_group(0, tt)
            deferred[0] = [(emit_qk_group, (0, mt)) for mt in LATE_MTS]

            from collections import deque
            psb_q = deque()   # psB-only work, consumed inside scores windows

            def psb_fill(n):
                for _ in range(n):
                    if psb_q:
                        f, a = psb_q.popleft()
                        f(*a)

            for b in range(BL):
                nxt = []
                if b >= 1:
                    # transposes(b-1) are psB work -> fill head-0 windows;
                    # proj(b-1) joins the psA work stream
                    ots[b - 1] = ot_pool.tile([128, KT, N], BF, name="oT",
                                              uniquify=True)
                    for qt in range(TT):
                        psb_q.append((emit_transpose, (b - 1, qt, False)))
                    nxt += [(emit_proj, (b - 1, qt)) for qt in range(TT)]
                if b + 1 < BL:
                    load_x(b + 1)
                    nxt += [(emit_qk_group, (b + 1, mt)) for mt in EARLY_MTS]
                    nxt += [(emit_v_group, (b + 1, tt)) for tt in range(TT)]
                    deferred[b + 1] = [(emit_qk_group, (b + 1, mt))
                                       for mt in LATE_MTS]
                for hh in range(NRES, min(NRES + 3, NH)):
                    prefetch_bias(b, hh)
                state = {"ni": 0, "quota": 0}
                mine = deferred.pop(b, [])

                def fill(n):
                    take = min(n, state["quota"], len(nxt) - state["ni"])
                    for _ in range(take):
                        f, a = nxt[state["ni"]]
                        f(*a)
                        state["ni"] += 1
                        state["quota"] -= 1

                for h in range(NH):
                    if NRES + 3 <= h + 3 < NH:
                        prefetch_bias(b, h + 3)
                    state["quota"] = (len(nxt) * (h + 1)) // NH - state["ni"]
                    expS = emit_scores(b, h, psb_fill)
                    if h < len(mine):
                        f, a = mine[h]
                        f(*a)
                    fill(len(nxt))
                    # this head's P@V runs inside the NEXT head's scores
                    # windows (it only needs psB slots there)
                    for qt in range(TT):
                        psb_q.append((emit_pv_piece, (b, h, expS, qt)))
                while psb_q:
                    f, a = psb_q.popleft()
                    f(*a)
                state["quota"] = len(nxt)
                fill(len(nxt))
            for f, a in tp_items(BL - 1, tail=True):
                f(*a)
    nc.finalize()
    return nc


_NC_CACHE = {}


def _get_nc():
    if "nc" not in _NC_CACHE:
        _NC_CACHE["nc"] = _build_nc()
    return _NC_CACHE["nc"]


def _q8(a):
    return np.asarray(a, np.float32).astype(F8)


def _prep_shared(qkv_w, q_bias, rpb_table, proj_w, rel_index):
    qkv_w = np.asarray(qkv_w, dtype=np.float32)
    Wq = qkv_w[:D].T * (SCALE * SQ)          # [768, 768] in-dim major
    Wk = qkv_w[D:2 * D].T * SK
    Wv = qkv_w[2 * D:].T * SV
    qk = np.concatenate([Wq, Wk], axis=1)     # [768, 1536]
    qk_hi = _q8(qk)
    qk_lo = _q8(qk - qk_hi.astype(np.float32))
    v_hi = _q8(Wv)
    v_lo = _q8(Wv - v_hi.astype(np.float32))
    # per-mt interleave [hi_mt | lo_mt] so a small leading DMA covers the
    # first matmul groups
    qk_il = np.empty((D, 3072), dtype=qk_hi.dtype)
    for mt in range(12):
        qk_il[:, 256 * mt:256 * mt + 128] = qk_hi[:, 128 * mt:128 * mt + 128]
        qk_il[:, 256 * mt + 128:256 * mt + 256] = \
            qk_lo[:, 128 * mt:128 * mt + 128]
    w8qk = np.ascontiguousarray(
        qk_il.reshape(KT, 128, 3072).transpose(1, 0, 2))   # [128, KT, 3072]
    w8v = np.ascontiguousarray(
        np.concatenate([v_hi, v_lo], axis=1)
        .reshape(KT, 128, 1536).transpose(1, 0, 2))        # [128, KT, 1536]

    wp = np.ascontiguousarray(
        np.asarray(proj_w, np.float32).T.reshape(KT, 128, D)
        .transpose(1, 0, 2)).astype(BF16)

    qb = np.zeros((128, 12), np.float32)
    qb_scaled = (np.asarray(q_bias, np.float32) * (SCALE * SQ)).reshape(KT, 128)
    qb[:, :KT] = qb_scaled.T

    rb = np.asarray(rpb_table, np.float32)[
        np.asarray(rel_index).reshape(-1)].reshape(N, N, NH)   # [q, k, h]
    M = np.exp(rb).transpose(2, 1, 0)                          # [h, k, q]
    Mp = np.zeros((NH, TT * 128, N), np.float32)
    Mp[:, :N] = M
    arr = Mp.reshape(NH, TT, 128, N).transpose(0, 2, 1, 3).astype(BF16)
    bres = np.ascontiguousarray(arr[:NRES].transpose(1, 0, 2, 3))
    bstr = np.ascontiguousarray(arr[NRES:])
    return w8qk, w8v, wp, qb, bres, bstr


def _make_in_maps(inputs):
    x = np.asarray(inputs["x"], dtype=np.float32)
    w8qk, w8v, wp, qb, bres, bstr = _prep_shared(
        inputs["qkv_w"], inputs["q_bias"], inputs["rpb_table"],
        inputs["proj_w"], inputs["rel_index"])

    in_maps = []
    for i in range(NCORES):
        xsl = x[i * BL:(i + 1) * BL]                    # [BL, N, D]
        xT = np.zeros((BL, D, NP), np.float32)
        xT[:, :, :N] = xsl.transpose(0, 2, 1)
        x8 = xT.astype(F8)
        xr8 = (xT - x8.astype(np.float32)).astype(F8)
        x8 = np.ascontiguousarray(
            x8.reshape(BL, KT, 128, NP).transpose(0, 2, 1, 3))
        xr8 = np.ascontiguousarray(
            xr8.reshape(BL, KT, 128, NP).transpose(0, 2, 1, 3))
        in_maps.append({
            "x8": x8, "xr8": xr8, "w8qk": w8qk, "w8v": w8v, "wp": wp,
            "qkvb": qb, "bres": bres, "bstr": bstr,
        })
    return in_maps


def _finish(inputs, res):
    out = np.concatenate(
        [np.asarray(res.results[i]["out"], np.float32) for i in range(NCORES)],
        axis=0)
    row = (np.asarray(inputs["v_bias"], np.float32)
           @ np.asarray(inputs["proj_w"], np.float32).T
           + np.asarray(inputs["proj_b"], np.float32))
    out += row[None, None, :]
    return np.ascontiguousarray(out)


def kernel(**inputs):
    in_maps = _make_in_maps(inputs)
    nc = _get_nc()
    res = run_bass_kernel_spmd(nc, in_maps, core_ids=list(range(NCORES)))
    return _finish(inputs, res)


def kernel_traced(**inputs):
    """Like kernel() but also returns (out, BassKernelResults with profile)."""
    in_maps = _make_in_maps(inputs)
    nc = _get_nc()
    res = run_bass_kernel_spmd(nc, in_maps, core_ids=list(range(NCORES)),
                               trace=True)
    return _finish(inputs, res), res


# revision 49
# speedup vs baseline: 1.3566x; 1.0291x over previous
"""BEiT-style windowed attention (B=32, N=577, D=768, 12 heads) on 8 TRN2 cores.

Strategy: pure data-parallel over batch (4 batch elements per core, no
collectives). qkv projection in fp8e4 DoubleRow matmuls with 3-term residual
compensation (hi@hi + lo@hi + hi@lo) for bf16-level accuracy at 1.33x bf16
speed; scores / P@V / out-proj in bf16 (fp8 there fails the 2e-2 gate:
logit-error rms transfers ~1:1 into the output metric). Softmax is unsafe-exp
with the scale constants folded into the fp8 weight encodings
(q: SQ*softmax_scale, k: SK, v: SV) and unfolded via the exp() scale immediate
and the final evacuation.

Per-core dataflow per batch element:
  qkT [d,tok]  = 9 fp8-DR matmuls per 128-col block (x8/xr8 vs W8hi/W8lo)
  v   [tok,d]  = same, transposed roles; kept at SV*v (ones col appended)
  S.T [k,q]    = kT(T) @ qT in bf16 (K=64)
  praw         = exp(S.T * 1/(SQ*SK))  on scalar engine
  P            = praw * exp(rel_bias).T   (DVE / GpSimd split)
  O_un [q,65]  = P(T) @ v_aug  (col 64 = rowsum)
  O            = O_un[:, :64] * recip(rowsum)   (= SV * O_norm)
  OT           = PE transpose, evacuated on DVE
  out  [tok,d] = OT(T) @ W_projT * 1/SV  -> bf16 -> HBM

v-bias and proj-bias are folded into a host-side row add (out += v_bias @
proj_w.T + proj_b); q-bias is applied in the qk evacuation (per-partition).

Issue order is software-pipelined: batch b+1's qkv matmul groups are
interleaved into batch b's attention heads so the PE never idles waiting on
exp/mul, and the second half of the bias table is streamed from DRAM (first
half resident) to fit SBUF.
"""

import numpy as np
import ml_dtypes

import concourse.bass as bass
import concourse.tile as tile
from concourse import bacc
from concourse import mybir
from concourse.bass_utils import run_bass_kernel_spmd
from concourse.masks import make_identity

B, N, D = 32, 577, 768
NH, DH = 12, 64
NCORES = 8
BL = B // NCORES            # 4 batch elements per core
SCALE = DH ** -0.5
KT = D // 128                # 6 contraction tiles over D
TT = (N + 127) // 128        # 5 token tiles (4x128 + 65)
NP = 592                     # fp8 x token pad (DoubleRow slice stride % 16)
SQ, SK, SV = 256.0, 32.0, 64.0   # fp8 scale folds for q / k / v
CW = 4608                    # w8 cols: qk-hi 1536 | qk-lo 1536 | v-hi 768 | v-lo 768
NRES = 0                     # bias-table heads resident in SBUF; rest streamed

BF16 = ml_dtypes.bfloat16
F8 = ml_dtypes.float8_e4m3   # TRN e4m3 (max normal 240)

F32 = mybir.dt.float32
BF = mybir.dt.bfloat16
E4 = mybir.dt.float8e4
DR = mybir.MatmulPerfMode.DoubleRow
ADD = mybir.AluOpType.add
MULT = mybir.AluOpType.mult


def tok_m(t):
    return min(128, N - 128 * t)


def _build_nc():
    nc = bacc.Bacc()

    x8_d = nc.declare_dram_parameter("x8", [BL, 128, KT, NP], E4, isOutput=False)
    xr8_d = nc.declare_dram_parameter("xr8", [BL, 128, KT, NP], E4, isOutput=False)
    w8qk_d = nc.declare_dram_parameter("w8qk", [128, KT, 3072], E4, isOutput=False)
    w8v_d = nc.declare_dram_parameter("w8v", [128, KT, 1536], E4, isOutput=False)
    wp_d = nc.declare_dram_parameter("wp", [128, KT, D], BF, isOutput=False)
    qkvb_d = nc.declare_dram_parameter("qkvb", [128, 12], F32, isOutput=False)
    bstr_d = nc.declare_dram_parameter("bstr", [NH - NRES, 128, TT, N], BF,
                                       isOutput=False)
    out_d = nc.declare_dram_parameter("out", [BL, N, D], BF, isOutput=True)

    Exp = mybir.ActivationFunctionType.Exp
    QCH_DR = [(0, 256), (256, 256), (512, N - 512)]   # qk DR out chunks
    VCH_DR = [(0, 256), (256, 256), (512, 256)]       # v DR out chunks
    SCH = [(0, 512), (512, N - 512)]                  # scores bf16 chunks
    DCH = [(0, 512), (512, 256)]                      # proj bf16 chunks

    with tile.TileContext(nc) as tc:
        with (
            tc.tile_pool(name="singles", bufs=1) as singles,
            tc.tile_pool(name="xp", bufs=2) as x_pool,
            tc.tile_pool(name="qktp", bufs=2) as qkt_pool,
            tc.tile_pool(name="vp", bufs=2) as v_pool,
            tc.tile_pool(name="bstrp", bufs=4) as bstr_pool,
            tc.tile_pool(name="prawp", bufs=4) as praw_pool,
            tc.tile_pool(name="expsp", bufs=15) as exps_pool,
            tc.tile_pool(name="op", bufs=2) as o_pool,
            tc.tile_pool(name="otp", bufs=2) as ot_pool,
            tc.tile_pool(name="outp", bufs=2) as out_pool,
            tc.tile_pool(name="smallp", bufs=4) as small_pool,
            tc.tile_pool(name="psA", bufs=3, space="PSUM") as psA,
            tc.tile_pool(name="psB", bufs=2, space="PSUM") as psB,
        ):
            # ---- one-time loads (ordered so batch-0 qk work starts ASAP:
            # x first, then qk weight cols; v cols / bias / proj stream in
            # behind while the first qk matmul groups run) ----
            x0 = None
            w8qk = singles.tile([128, KT, 3072], E4)
            w8v = singles.tile([128, KT, 1536], E4)
            qkvb = singles.tile([128, 12], F32)
            wproj = singles.tile([128, KT, D], BF)
            ident = singles.tile([128, 128], BF)

            xs = {}      # b -> (x8 tile, xr8 tile)
            qkts = {}    # b -> qkT tile [128, 12, N] bf16
            vs = {}      # b -> v_sb tile (strided view with ones col)
            os_ = {}     # b -> o_sb tile
            bias_tiles = {}   # (b, h) -> streamed bias tile

            def load_x(b, split=False):
                x8 = x_pool.tile([128, KT, NP], E4, name="x8t", tag="x8")
                xr8 = x_pool.tile([128, KT, NP], E4, name="xr8t", tag="xr8")
                nc.sync.dma_start(out=x8, in_=x8_d[b])
                if not split:
                    nc.sync.dma_start(out=xr8, in_=xr8_d[b])
                xs[b] = (x8, xr8)
                return xr8

            def prefetch_bias(b, h):
                t = bstr_pool.tile([128, TT, N], BF, name="biash")
                nc.sync.dma_start(out=t, in_=bstr_d[h - NRES])
                bias_tiles[(b, h)] = t

            def bias_ap(b, h):
                return bias_tiles[(b, h)]

            def emit_qk_group(b, mt):
                """One 128-col block of the q/k projection: 27 fp8-DR matmuls."""
                x8, xr8 = xs[b]
                if b not in qkts:
                    qkts[b] = qkt_pool.tile([128, 2 * KT, N], BF, name="qkT")
                ps = psA.tile([128, N], F32, name="ps_qk", tag="A")
                for c0, w in QCH_DR:
                    idx = 0
                    for coff, xt in ((0, x8), (128, x8), (0, xr8)):
                        for kp in range(3):
                            nc.tensor.matmul(
                                ps[:, c0:c0 + w],
                                w8qk[:, 2 * kp:2 * kp + 2,
                                     coff + 256 * mt:coff + 256 * mt + 128],
                                xt[:, 2 * kp:2 * kp + 2, c0:c0 + w],
                                start=(idx == 0), stop=(idx == 8),
                                perf_mode=DR,
                            )
                            idx += 1
                nc.vector.tensor_scalar(
                    out=qkts[b][:, mt, :], in0=ps[:, :N],
                    scalar1=qkvb[:, mt:mt + 1], scalar2=None, op0=ADD,
                )

            def emit_v_group(b, tt):
                """One 128-token block of the v projection (kept at SV*v)."""
                x8, xr8 = xs[b]
                if b not in vs:
                    v_sb = v_pool.tile([128, TT, NH * 65], BF, name="v_sb")
                    v_str = v_sb.rearrange("p t (h c) -> p t h c", c=65)
                    nc.vector.memset(v_str[:, :, :, 64:65], 1.0)
                    vs[b] = v_str
                v_str = vs[b]
                m = tok_m(tt)
                ps = psA.tile([128, D], F32, name="ps_v", tag="A")
                for c0, w in VCH_DR:
                    idx = 0
                    for xt, coff in ((x8, 0), (xr8, 0), (x8, 768)):
                        for kp in range(3):
                            nc.tensor.matmul(
                                ps[:m, c0:c0 + w],
                                xt[:, 2 * kp:2 * kp + 2, 128 * tt:128 * tt + m],
                                w8v[:, 2 * kp:2 * kp + 2, coff + c0:coff + c0 + w],
                                start=(idx == 0), stop=(idx == 8),
                                perf_mode=DR,
                            )
                            idx += 1
                nc.vector.tensor_scalar(
                    out=v_str[:m, tt, :, 0:64],
                    in0=ps[:m, :].rearrange("p (h c) -> p h c", c=64),
                    scalar1=1.0, scalar2=None, op0=MULT,
                )

            def emit_scores(b, h, fill=None):
                """S.T tiles -> exp -> bias-mul for one head; returns expS tiles.

                `fill(n)` emits up to n deferred PE work items; called between
                score tiles so the PE has queued work while psA slots recycle
                at exp() speed.
                """
                qkT = qkts[b]
                po = 64 * (h % 2)
                qT = qkT[po:po + 64, h // 2, :]
                kTh = qkT[po:po + 64, KT + h // 2, :]
                bh = bias_ap(b, h)
                expS = [exps_pool.tile([128, N], BF, name="expS", tag="es")
                        for _ in range(TT)]
                for kt in range(TT):
                    if fill is not None and kt >= 2:
                        # window where scores wait on exp() draining a psA
                        # slot; only psB-based work (P@V of the previous head,
                        # transposes) can actually run here
                        fill(2 if kt > 2 else 1)
                    km = tok_m(kt)
                    ps_s = psA.tile([128, N], F32, name="ps_s", tag="A")
                    for c0, w in SCH:
                        nc.tensor.matmul(
                            ps_s[:km, c0:c0 + w],
                            kTh[:, 128 * kt:128 * kt + km],
                            qT[:, c0:c0 + w],
                            start=True, stop=True,
                        )
                    praw = praw_pool.tile([128, N], BF)
                    nc.scalar.activation(praw[:km, :], ps_s[:km, :], Exp,
                                         scale=1.0 / (SQ * SK))
                    eng = nc.gpsimd
                    eng.tensor_mul(expS[kt][:km, :], praw[:km, :], bh[:km, kt, :])
                return expS

            def emit_pv_piece(b, h, expS, qt):
                if b not in os_:
                    os_[b] = o_pool.tile([128, TT, D], BF, name="o_sb")
                o_sb = os_[b]
                v_str = vs[b]
                # kt order starts at TT-1: the wait on the last bias-mul is
                # consolidated at one fillable point instead of stalling each
                # qt chain mid-accumulation
                kt_order = [TT - 1] + list(range(TT - 1))
                qm = tok_m(qt)
                ps_o = psB.tile([128, 128], F32, name="ps_o", tag="B")
                for i, kt in enumerate(kt_order):
                    km = tok_m(kt)
                    nc.tensor.matmul(
                        ps_o[:qm, :65],
                        expS[kt][:km, 128 * qt:128 * qt + qm],
                        v_str[:km, kt, h, :],
                        start=(i == 0), stop=(i == TT - 1),
                    )
                rcp = small_pool.tile([128, 1], F32)
                nc.vector.reciprocal(rcp[:qm], ps_o[:qm, 64:65])
                nc.vector.tensor_mul(
                    o_sb[:qm, qt, 64 * h:64 * h + 64],
                    ps_o[:qm, 0:64],
                    rcp[:qm, 0:1].to_broadcast([qm, 64]),
                )

            def emit_transpose(b, qt, tail=False):
                oT = ots[b]
                qm = tok_m(qt)
                o_sb = os_[b]
                for dt_ in range(KT):
                    ps_t = psB.tile([128, 128], BF, name="ps_t", tag="B")
                    nc.tensor.transpose(
                        ps_t[:, :qm],
                        o_sb[:qm, qt, 128 * dt_:128 * (dt_ + 1)],
                        ident[:qm, :qm],
                    )
                    if tail and dt_ % 2:
                        # final batch: Act is idle, split the psum drain
                        nc.scalar.copy(oT[:, dt_, 128 * qt:128 * qt + qm],
                                       ps_t[:, :qm])
                    else:
                        nc.vector.tensor_scalar(
                            out=oT[:, dt_, 128 * qt:128 * qt + qm],
                            in0=ps_t[:, :qm], scalar1=1.0, scalar2=None,
                            op0=MULT,
                        )

            def emit_proj(b, tt):
                oT = ots[b]
                m = tok_m(tt)
                ps = psA.tile([128, D], F32, name="ps_p", tag="A")
                out_sb = out_pool.tile([128, D], BF)
                # chunk-major so the first chunk's evacuation overlaps the
                # second chunk's matmuls
                for c0, w in DCH:
                    for kt in range(KT):
                        nc.tensor.matmul(
                            ps[:m, c0:c0 + w],
                            oT[:, kt, 128 * tt:128 * tt + m],
                            wproj[:, kt, c0:c0 + w],
                            start=(kt == 0), stop=(kt == KT - 1),
                        )
                    nc.vector.tensor_scalar(
                        out=out_sb[:m, c0:c0 + w], in0=ps[:m, c0:c0 + w],
                        scalar1=1.0 / SV, scalar2=None, op0=MULT,
                    )
                nc.sync.dma_start(out=out_d[b, 128 * tt:128 * tt + m, :],
                                  in_=out_sb[:m, :])

            # ---- software-pipelined emission ----
            # attn(b) is interleaved with qkv(b+1) and transpose+proj(b-1) so
            # the PE stream has fill work while exp/mul latencies drain.
            xr8_0 = load_x(0, split=True)
            nc.sync.dma_start(out=w8qk[:, :, 0:768], in_=w8qk_d[:, :, 0:768])
            nc.sync.dma_start(out=xr8_0, in_=xr8_d[0])
            nc.sync.dma_start(out=qkvb, in_=qkvb_d[:])
            nc.sync.dma_start(out=w8v, in_=w8v_d[:])
            nc.sync.dma_start(out=w8qk[:, :, 768:3072],
                              in_=w8qk_d[:, :, 768:3072])
            make_identity(nc, ident)

            ots = {}

            def tp_items(b, tail=False):
                ots[b] = ot_pool.tile([128, KT, N], BF, name="oT",
                                      uniquify=True)
                items = []
                for qt in range(TT):
                    items.append((emit_transpose, (b, qt, tail)))
                    items.append((emit_proj, (b, qt)))
                return items

            # qk-groups for mts needed only from head 4 on are deferred into
            # the OWNING batch's early attention heads, so every attention
            # phase (including the last batch's) has PE fill work.
            LATE_MTS = [2, KT + 2, 3, KT + 3, 4, KT + 4, 5, KT + 5]
            EARLY_MTS = [0, 1, KT, KT + 1]
            deferred = {}   # b -> list of (emit fn, args) pinned to heads 0..3

            for mt in EARLY_MTS:
                emit_qk_group(0, mt)
            for tt in range(TT):
                emit_v_group(0, tt)
            deferred[0] = [(emit_qk_group, (0, mt)) for mt in LATE_MTS]

            from collections import deque
            psb_q = deque()   # psB-only work, consumed inside scores windows

            def psb_fill(n):
                for _ in range(n):
                    if psb_q:
                        f, a = psb_q.popleft()
                        f(*a)

            for b in range(BL):
                nxt = []
                if b == 1:
                    nc.sync.dma_start(out=wproj, in_=wp_d[:])
                if b >= 1:
                    # transposes(b-1) are psB work -> fill head-0 windows;
                    # proj(b-1) joins the psA work stream
                    ots[b - 1] = ot_pool.tile([128, KT, N], BF, name="oT",
                                              uniquify=True)
                    for qt in range(TT):
                        psb_q.append((emit_transpose, (b - 1, qt, False)))
                    nxt += [(emit_proj, (b - 1, qt)) for qt in range(TT)]
                if b + 1 < BL:
                    load_x(b + 1)
                    nxt += [(emit_qk_group, (b + 1, mt)) for mt in EARLY_MTS]
                    nxt += [(emit_v_group, (b + 1, tt)) for tt in range(TT)]
                    deferred[b + 1] = [(emit_qk_group, (b + 1, mt))
                                       for mt in LATE_MTS]
                for hh in range(NRES, min(NRES + 4, NH)):
                    prefetch_bias(b, hh)
                state = {"ni": 0, "quota": 0}
                mine = deferred.pop(b, [])

                def fill(n):
                    take = min(n, state["quota"], len(nxt) - state["ni"])
                    for _ in range(take):
                        f, a = nxt[state["ni"]]
                        f(*a)
                        state["ni"] += 1
                        state["quota"] -= 1

                for h in range(NH):
                    if NRES + 4 <= h + 4 < NH:
                        prefetch_bias(b, h + 4)
                    state["quota"] = (len(nxt) * (h + 1)) // NH - state["ni"]
                    expS = emit_scores(b, h, psb_fill)
                    if h < len(mine):
                        f, a = mine[h]
                        f(*a)
                    fill(len(nxt))
                    # this head's P@V runs inside the NEXT head's scores
                    # windows (it only needs psB slots there)
                    for qt in range(TT):
                        psb_q.append((emit_pv_piece, (b, h, expS, qt)))
                while psb_q:
                    f, a = psb_q.popleft()
                    f(*a)
                state["quota"] = len(nxt)
                fill(len(nxt))
            for f, a in tp_items(BL - 1, tail=True):
                f(*a)
    nc.finalize()
    return nc


_NC_CACHE = {}


def _get_nc():
    if "nc" not in _NC_CACHE:
        _NC_CACHE["nc"] = _build_nc()
    return _NC_CACHE["nc"]


def _q8(a):
    return np.asarray(a, np.float32).astype(F8)


def _prep_shared(qkv_w, q_bias, rpb_table, proj_w, rel_index):
    qkv_w = np.asarray(qkv_w, dtype=np.float32)
    Wq = qkv_w[:D].T * (SCALE * SQ)          # [768, 768] in-dim major
    Wk = qkv_w[D:2 * D].T * SK
    Wv = qkv_w[2 * D:].T * SV
    qk = np.concatenate([Wq, Wk], axis=1)     # [768, 1536]
    qk_hi = _q8(qk)
    qk_lo = _q8(qk - qk_hi.astype(np.float32))
    v_hi = _q8(Wv)
    v_lo = _q8(Wv - v_hi.astype(np.float32))
    # per-mt interleave [hi_mt | lo_mt] so a small leading DMA covers the
    # first matmul groups
    qk_il = np.empty((D, 3072), dtype=qk_hi.dtype)
    for mt in range(12):
        qk_il[:, 256 * mt:256 * mt + 128] = qk_hi[:, 128 * mt:128 * mt + 128]
        qk_il[:, 256 * mt + 128:256 * mt + 256] = \
            qk_lo[:, 128 * mt:128 * mt + 128]
    w8qk = np.ascontiguousarray(
        qk_il.reshape(KT, 128, 3072).transpose(1, 0, 2))   # [128, KT, 3072]
    w8v = np.ascontiguousarray(
        np.concatenate([v_hi, v_lo], axis=1)
        .reshape(KT, 128, 1536).transpose(1, 0, 2))        # [128, KT, 1536]

    wp = np.ascontiguousarray(
        np.asarray(proj_w, np.float32).T.reshape(KT, 128, D)
        .transpose(1, 0, 2)).astype(BF16)

    qb = np.zeros((128, 12), np.float32)
    qb_scaled = (np.asarray(q_bias, np.float32) * (SCALE * SQ)).reshape(KT, 128)
    qb[:, :KT] = qb_scaled.T

    rb = np.asarray(rpb_table, np.float32)[
        np.asarray(rel_index).reshape(-1)].reshape(N, N, NH)   # [q, k, h]
    M = np.exp(rb).transpose(2, 1, 0)                          # [h, k, q]
    Mp = np.zeros((NH, TT * 128, N), np.float32)
    Mp[:, :N] = M
    arr = Mp.reshape(NH, TT, 128, N).transpose(0, 2, 1, 3).astype(BF16)
    bstr = np.ascontiguousarray(arr[NRES:])
    return w8qk, w8v, wp, qb, bstr


def _make_in_maps(inputs):
    x = np.asarray(inputs["x"], dtype=np.float32)
    w8qk, w8v, wp, qb, bstr = _prep_shared(
        inputs["qkv_w"], inputs["q_bias"], inputs["rpb_table"],
        inputs["proj_w"], inputs["rel_index"])

    in_maps = []
    for i in range(NCORES):
        xsl = x[i * BL:(i + 1) * BL]                    # [BL, N, D]
        xT = np.zeros((BL, D, NP), np.float32)
        xT[:, :, :N] = xsl.transpose(0, 2, 1)
        x8 = xT.astype(F8)
        xr8 = (xT - x8.astype(np.float32)).astype(F8)
        x8 = np.ascontiguousarray(
            x8.reshape(BL, KT, 128, NP).transpose(0, 2, 1, 3))
        xr8 = np.ascontiguousarray(
            xr8.reshape(BL, KT, 128, NP).transpose(0, 2, 1, 3))
        in_maps.append({
            "x8": x8, "xr8": xr8, "w8qk": w8qk, "w8v": w8v, "wp": wp,
            "qkvb": qb, "bstr": bstr,
        })
    return in_maps


def _finish(inputs, res):
    out = np.concatenate(
        [np.asarray(res.results[i]["out"], np.float32) for i in range(NCORES)],
        axis=0)
    row = (np.asarray(inputs["v_bias"], np.float32)
           @ np.asarray(inputs["proj_w"], np.float32).T
           + np.asarray(inputs["proj_b"], np.float32))
    out += row[None, None, :]
    return np.ascontiguousarray(out)


def kernel(**inputs):
    in_maps = _make_in_maps(inputs)
    nc = _get_nc()
    res = run_bass_kernel_spmd(nc, in_maps, core_ids=list(range(NCORES)))
    return _finish(inputs, res)


def kernel_traced(**inputs):
    """Like kernel() but also returns (out, BassKernelResults with profile)."""
    in_maps = _make_in_maps(inputs)
    nc = _get_nc()
    res = run_bass_kernel_spmd(nc, in_maps, core_ids=list(range(NCORES)),
                               trace=True)
    return _finish(inputs, res), res


# revision 61
# speedup vs baseline: 1.3855x; 1.0214x over previous
"""BEiT-style windowed attention (B=32, N=577, D=768, 12 heads) on 8 TRN2 cores.

Strategy: pure data-parallel over batch (4 batch elements per core, no
collectives). qkv projection in fp8e4 DoubleRow matmuls with 3-term residual
compensation (hi@hi + lo@hi + hi@lo) for bf16-level accuracy at 1.33x bf16
speed; scores / P@V / out-proj in bf16 (fp8 there fails the 2e-2 gate:
logit-error rms transfers ~1:1 into the output metric). Softmax is unsafe-exp
with the scale constants folded into the fp8 weight encodings
(q: SQ*softmax_scale, k: SK, v: SV) and unfolded via the exp() scale immediate
and the final evacuation.

Per-core dataflow per batch element:
  qkT [d,tok]  = 9 fp8-DR matmuls per 128-col block (x8/xr8 vs W8hi/W8lo)
  v   [tok,d]  = same, transposed roles; kept at SV*v (ones col appended)
  S.T [k,q]    = kT(T) @ qT in bf16 (K=64)
  praw         = exp(S.T * 1/(SQ*SK))  on scalar engine
  P            = praw * exp(rel_bias).T   (DVE / GpSimd split)
  O_un [q,65]  = P(T) @ v_aug  (col 64 = rowsum)
  O            = O_un[:, :64] * recip(rowsum)   (= SV * O_norm)
  OT           = XBAR DMA transpose (SBUF->SBUF)
  out  [tok,d] = OT(T) @ W_projT * 1/SV  -> bf16 -> HBM

v-bias and proj-bias are folded into a host-side row add (out += v_bias @
proj_w.T + proj_b); q-bias is applied in the qk evacuation (per-partition).

Issue order is software-pipelined: batch b+1's qkv matmul groups are
interleaved into batch b's attention heads so the PE never idles waiting on
exp/mul, and the second half of the bias table is streamed from DRAM (first
half resident) to fit SBUF.
"""

import numpy as np
import ml_dtypes

import concourse.bass as bass
import concourse.tile as tile
from concourse import bacc
from concourse import mybir
from concourse.bass_utils import run_bass_kernel_spmd
from concourse.masks import make_identity

B, N, D = 32, 577, 768
NH, DH = 12, 64
NCORES = 8
BL = B // NCORES            # 4 batch elements per core
SCALE = DH ** -0.5
KT = D // 128                # 6 contraction tiles over D
TT = (N + 127) // 128        # 5 token tiles (4x128 + 65)
NP = 592                     # fp8 x token pad (DoubleRow slice stride % 16)
SQ, SK, SV = 256.0, 32.0, 64.0   # fp8 scale folds for q / k / v
NRES = 0                     # bias-table heads resident in SBUF; rest streamed

BF16 = ml_dtypes.bfloat16
F8 = ml_dtypes.float8_e4m3   # TRN e4m3 (max normal 240)

F32 = mybir.dt.float32
BF = mybir.dt.bfloat16
E4 = mybir.dt.float8e4
DR = mybir.MatmulPerfMode.DoubleRow
ADD = mybir.AluOpType.add
MULT = mybir.AluOpType.mult


# weight-column block order = first-use order during the pipelined emission
MT_ORDER = [0, 6, 1, 7, 2, 8, 3, 9, 4, 10, 5, 11]
MT_POS = {m: i for i, m in enumerate(MT_ORDER)}


def tok_m(t):
    return min(128, N - 128 * t)


def _build_nc():
    nc = bacc.Bacc()

    x8_d = nc.declare_dram_parameter("x8", [BL, 128, KT, NP], E4, isOutput=False)
    xr8_d = nc.declare_dram_parameter("xr8", [BL, 128, KT, NP], E4, isOutput=False)
    w8qk_d = nc.declare_dram_parameter("w8qk", [128, KT, 3072], E4, isOutput=False)
    w8v_d = nc.declare_dram_parameter("w8v", [128, KT, 1536], E4, isOutput=False)
    wp_d = nc.declare_dram_parameter("wp", [128, KT, D], BF, isOutput=False)
    qkvb_d = nc.declare_dram_parameter("qkvb", [128, 12], F32, isOutput=False)
    bstr_d = nc.declare_dram_parameter("bstr", [NH - NRES, 128, TT, N], BF,
                                       isOutput=False)
    out_d = nc.declare_dram_parameter("out", [BL, N, D], BF, isOutput=True)

    Exp = mybir.ActivationFunctionType.Exp
    QCH_DR = [(0, 256), (256, 256), (512, N - 512)]   # qk DR out chunks
    VCH_DR = [(0, 256), (256, 256), (512, 256)]       # v DR out chunks
    SCH = [(0, 512), (512, N - 512)]                  # scores bf16 chunks
    DCH = [(0, 512), (512, 256)]                      # proj bf16 chunks

    with tile.TileContext(nc) as tc:
        with (
            tc.tile_pool(name="singles", bufs=1) as singles,
            tc.tile_pool(name="xp", bufs=2) as x_pool,
            tc.tile_pool(name="qktp", bufs=2) as qkt_pool,
            tc.tile_pool(name="vp", bufs=2) as v_pool,
            tc.tile_pool(name="bstrp", bufs=4) as bstr_pool,
            tc.tile_pool(name="prawp", bufs=4) as praw_pool,
            tc.tile_pool(name="expsp", bufs=15) as exps_pool,
            tc.tile_pool(name="op", bufs=2) as o_pool,
            tc.tile_pool(name="otp", bufs=2) as ot_pool,
            tc.tile_pool(name="outp", bufs=2) as out_pool,
            tc.tile_pool(name="smallp", bufs=4) as small_pool,
            tc.tile_pool(name="psA", bufs=3, space="PSUM") as psA,
            tc.tile_pool(name="psB", bufs=2, space="PSUM") as psB,
        ):
            # ---- one-time loads (ordered so batch-0 qk work starts ASAP:
            # x first, then qk weight cols; v cols / bias / proj stream in
            # behind while the first qk matmul groups run) ----
            w8qk = singles.tile([128, KT, 3072], E4)
            w8v = singles.tile([128, KT, 1536], E4)
            qkvb = singles.tile([128, 12], F32)
            wproj = singles.tile([128, KT, D], BF)
            ident = singles.tile([128, 128], BF)

            xs = {}      # b -> (x8 tile, xr8 tile)
            qkts = {}    # b -> qkT tile [128, 12, N] bf16
            vs = {}      # b -> v_sb tile (strided view with ones col)
            os_ = {}     # b -> o_sb tile
            bias_tiles = {}   # (b, h) -> streamed bias tile

            def load_x(b, split=False):
                x8 = x_pool.tile([128, KT, NP], E4, name="x8t", tag="x8")
                xr8 = x_pool.tile([128, KT, NP], E4, name="xr8t", tag="xr8")
                nc.sync.dma_start(out=x8, in_=x8_d[b])
                if not split:
                    nc.sync.dma_start(out=xr8, in_=xr8_d[b])
                xs[b] = (x8, xr8)
                return xr8

            def prefetch_bias(b, h):
                t = bstr_pool.tile([128, TT, N], BF, name="biash")
                nc.sync.dma_start(out=t, in_=bstr_d[h - NRES])
                bias_tiles[(b, h)] = t

            def bias_ap(b, h):
                return bias_tiles[(b, h)]

            def emit_qk_group(b, mt):
                """One 128-col block of the q/k projection: 27 fp8-DR matmuls."""
                x8, xr8 = xs[b]
                if b not in qkts:
                    qkts[b] = qkt_pool.tile([128, 2 * KT, N], BF, name="qkT")
                ps = psA.tile([128, N], F32, name="ps_qk", tag="G", bufs=1)
                cb = 256 * MT_POS[mt]
                for c0, w in QCH_DR:
                    idx = 0
                    for coff, xt in ((0, x8), (128, x8), (0, xr8)):
                        for kp in range(3):
                            nc.tensor.matmul(
                                ps[:, c0:c0 + w],
                                w8qk[:, 2 * kp:2 * kp + 2,
                                     coff + cb:coff + cb + 128],
                                xt[:, 2 * kp:2 * kp + 2, c0:c0 + w],
                                start=(idx == 0), stop=(idx == 8),
                                perf_mode=DR,
                            )
                            idx += 1
                nc.vector.tensor_scalar(
                    out=qkts[b][:, mt, :], in0=ps[:, :N],
                    scalar1=qkvb[:, mt:mt + 1], scalar2=None, op0=ADD,
                )

            def emit_v_group(b, tt):
                """One 128-token block of the v projection (kept at SV*v)."""
                x8, xr8 = xs[b]
                if b not in vs:
                    v_sb = v_pool.tile([128, TT, NH * 65], BF, name="v_sb")
                    v_str = v_sb.rearrange("p t (h c) -> p t h c", c=65)
                    nc.vector.memset(v_str[:, :, :, 64:65], 1.0)
                    vs[b] = v_str
                v_str = vs[b]
                m = tok_m(tt)
                ps = psA.tile([128, D], F32, name="ps_v", tag="G", bufs=1)
                for c0, w in VCH_DR:
                    idx = 0
                    for xt, coff in ((x8, 0), (xr8, 0), (x8, 768)):
                        for kp in range(3):
                            nc.tensor.matmul(
                                ps[:m, c0:c0 + w],
                                xt[:, 2 * kp:2 * kp + 2, 128 * tt:128 * tt + m],
                                w8v[:, 2 * kp:2 * kp + 2, coff + c0:coff + c0 + w],
                                start=(idx == 0), stop=(idx == 8),
                                perf_mode=DR,
                            )
                            idx += 1
                nc.vector.tensor_scalar(
                    out=v_str[:m, tt, :, 0:64],
                    in0=ps[:m, :].rearrange("p (h c) -> p h c", c=64),
                    scalar1=1.0, scalar2=None, op0=MULT,
                )

            def emit_scores(b, h, fill=None):
                """S.T tiles -> exp -> bias-mul for one head; returns expS tiles.

                `fill(n)` emits up to n deferred PE work items; called between
                score tiles so the PE has queued work while psA slots recycle
                at exp() speed.
                """
                qkT = qkts[b]
                po = 64 * (h % 2)
                qT = qkT[po:po + 64, h // 2, :]
                kTh = qkT[po:po + 64, KT + h // 2, :]
                bh = bias_ap(b, h)
                expS = [exps_pool.tile([128, N], BF, name="expS", tag="es")
                        for _ in range(TT)]
                for kt in range(TT):
                    if fill is not None and kt >= 2:
                        # window where scores wait on exp() draining a psA
                        # slot; only psB-based work (P@V of the previous head,
                        # transposes) can actually run here
                        fill(2 if kt > 2 else 1)
                    km = tok_m(kt)
                    ps_s = psA.tile([128, N], F32, name="ps_s", tag="A", bufs=2)
                    for c0, w in SCH:
                        nc.tensor.matmul(
                            ps_s[:km, c0:c0 + w],
                            kTh[:, 128 * kt:128 * kt + km],
                            qT[:, c0:c0 + w],
                            start=True, stop=True,
                        )
                    praw = praw_pool.tile([128, N], BF)
                    nc.scalar.activation(praw[:km, :], ps_s[:km, :], Exp,
                                         scale=1.0 / (SQ * SK))
                    eng = nc.gpsimd
                    eng.tensor_mul(expS[kt][:km, :], praw[:km, :], bh[:km, kt, :])
                return expS

            def emit_pv_piece(b, h, expS, qt):
                if b not in os_:
                    os_[b] = o_pool.tile([128, TT, D], BF, name="o_sb")
                o_sb = os_[b]
                v_str = vs[b]
                # kt order starts at TT-1: the wait on the last bias-mul is
                # consolidated at one fillable point instead of stalling each
                # qt chain mid-accumulation
                kt_order = [TT - 1] + list(range(TT - 1))
                qm = tok_m(qt)
                ps_o = psB.tile([128, 128], F32, name="ps_o", tag="B")
                for i, kt in enumerate(kt_order):
                    km = tok_m(kt)
                    nc.tensor.matmul(
                        ps_o[:qm, :65],
                        expS[kt][:km, 128 * qt:128 * qt + qm],
                        v_str[:km, kt, h, :],
                        start=(i == 0), stop=(i == TT - 1),
                    )
                rcp = small_pool.tile([128, 1], F32)
                nc.vector.reciprocal(rcp[:qm], ps_o[:qm, 64:65])
                nc.vector.tensor_mul(
                    o_sb[:qm, qt, 64 * h:64 * h + 64],
                    ps_o[:qm, 0:64],
                    rcp[:qm, 0:1].to_broadcast([qm, 64]),
                )

            def emit_transpose(b, qt):
                # XBAR DMA transpose: o_sb[q, 768] -> oT[d, kt, q-cols].
                # For qt4 the full 128 partitions are sent; garbage rows land
                # in oT cols >=577 which proj never reads.
                nc.sync.dma_start_transpose(
                    out=ots[b][:, :, 128 * qt:128 * (qt + 1)],
                    in_=os_[b][:, qt, :],
                )

            def emit_proj(b, tt):
                oT = ots[b]
                m = tok_m(tt)
                ps = psA.tile([128, D], F32, name="ps_p", tag="G", bufs=1)
                out_sb = out_pool.tile([128, D], BF)
                # chunk-major so the first chunk's evacuation overlaps the
                # second chunk's matmuls
                for c0, w in DCH:
                    for kt in range(KT):
                        nc.tensor.matmul(
                            ps[:m, c0:c0 + w],
                            oT[:, kt, 128 * tt:128 * tt + m],
                            wproj[:, kt, c0:c0 + w],
                            start=(kt == 0), stop=(kt == KT - 1),
                        )
                    nc.vector.tensor_scalar(
                        out=out_sb[:m, c0:c0 + w], in0=ps[:m, c0:c0 + w],
                        scalar1=1.0 / SV, scalar2=None, op0=MULT,
                    )
                nc.sync.dma_start(out=out_d[b, 128 * tt:128 * tt + m, :],
                                  in_=out_sb[:m, :])

            # ---- software-pipelined emission ----
            # attn(b) is interleaved with qkv(b+1) and transpose+proj(b-1) so
            # the PE stream has fill work while exp/mul latencies drain.
            xr8_0 = load_x(0, split=True)
            nc.sync.dma_start(out=w8qk[:, :, 0:1024], in_=w8qk_d[:, :, 0:1024])
            nc.sync.dma_start(out=xr8_0, in_=xr8_d[0])
            nc.sync.dma_start(out=w8qk[:, :, 1024:2048],
                              in_=w8qk_d[:, :, 1024:2048])
            nc.sync.dma_start(out=qkvb, in_=qkvb_d[:])
            nc.sync.dma_start(out=w8qk[:, :, 2048:3072],
                              in_=w8qk_d[:, :, 2048:3072])
            nc.sync.dma_start(out=w8v, in_=w8v_d[:])
            nc.sync.dma_start(out=wproj, in_=wp_d[:])
            make_identity(nc, ident)

            ots = {}

            def tp_items(b, tail=False):
                ots[b] = ot_pool.tile([128, KT, TT * 128], BF, name="oT",
                                      uniquify=True)
                items = []
                for qt in range(TT):
                    emit_transpose(b, qt)
                items += [(emit_proj, (b, qt)) for qt in range(TT)]
                return items

            # qk-groups for mts needed only from head 4 on are deferred into
            # the OWNING batch's early attention heads, so every attention
            # phase (including the last batch's) has PE fill work.
            LATE_MTS = [2, KT + 2, 3, KT + 3, 4, KT + 4, 5, KT + 5]
            EARLY_MTS = [0, 1, KT, KT + 1]
            deferred = {}   # b -> list of (emit fn, args) pinned to heads 0..3

            for mt in EARLY_MTS:
                emit_qk_group(0, mt)
            deferred[0] = [(emit_qk_group, (0, mt)) for mt in LATE_MTS]

            from collections import deque
            psb_q = deque()   # psB-only work, consumed inside scores windows

            def psb_fill(n):
                for _ in range(n):
                    if psb_q:
                        f, a = psb_q.popleft()
                        f(*a)

            for b in range(BL):
                nxt = []
                if b >= 1:
                    # transposes(b-1) are psB work -> fill head-0 windows;
                    # proj(b-1) joins the psA work stream
                    ots[b - 1] = ot_pool.tile([128, KT, TT * 128], BF,
                                              name="oT", uniquify=True)
                    for qt in range(TT):
                        emit_transpose(b - 1, qt)
                    nxt += [(emit_proj, (b - 1, qt)) for qt in range(TT)]
                if b + 1 < BL:
                    load_x(b + 1)
                    nxt += [(emit_qk_group, (b + 1, mt)) for mt in EARLY_MTS]
                    nxt += [(emit_v_group, (b + 1, tt)) for tt in range(TT)]
                    deferred[b + 1] = [(emit_qk_group, (b + 1, mt))
                                       for mt in LATE_MTS]
                for hh in range(NRES, min(NRES + 4, NH)):
                    prefetch_bias(b, hh)
                state = {"ni": 0, "quota": 0}
                mine = deferred.pop(b, [])

                def fill(n):
                    take = min(n, state["quota"], len(nxt) - state["ni"])
                    for _ in range(take):
                        f, a = nxt[state["ni"]]
                        f(*a)
                        state["ni"] += 1
                        state["quota"] -= 1

                for h in range(NH):
                    if NRES + 4 <= h + 4 < NH:
                        prefetch_bias(b, h + 4)
                    state["quota"] = (len(nxt) * (h + 1)) // NH - state["ni"]
                    expS = emit_scores(b, h, psb_fill)
                    if h < len(mine):
                        f, a = mine[h]
                        f(*a)
                    if b == 0 and h == 0:
                        # v(0) deadline is h1's windows; by now w8v has landed
                        for tt in range(TT):
                            emit_v_group(0, tt)
                    fill(len(nxt))
                    # this head's P@V runs inside the NEXT head's scores
                    # windows (it only needs psB slots there)
                    for qt in range(TT):
                        psb_q.append((emit_pv_piece, (b, h, expS, qt)))
                while psb_q:
                    f, a = psb_q.popleft()
                    f(*a)
                state["quota"] = len(nxt)
                fill(len(nxt))
            for f, a in tp_items(BL - 1, tail=True):
                f(*a)
    nc.finalize()
    return nc


_NC_CACHE = {}


def _get_nc():
    if "nc" not in _NC_CACHE:
        _NC_CACHE["nc"] = _build_nc()
    return _NC_CACHE["nc"]


def _q8(a):
    return np.asarray(a, np.float32).astype(F8)


def _prep_shared(qkv_w, q_bias, rpb_table, proj_w, rel_index):
    qkv_w = np.asarray(qkv_w, dtype=np.float32)
    Wq = qkv_w[:D].T * (SCALE * SQ)          # [768, 768] in-dim major
    Wk = qkv_w[D:2 * D].T * SK
    Wv = qkv_w[2 * D:].T * SV
    qk = np.concatenate([Wq, Wk], axis=1)     # [768, 1536]
    qk_hi = _q8(qk)
    qk_lo = _q8(qk - qk_hi.astype(np.float32))
    v_hi = _q8(Wv)
    v_lo = _q8(Wv - v_hi.astype(np.float32))
    # per-mt interleave [hi_mt | lo_mt] so a small leading DMA covers the
    # first matmul groups
    qk_il = np.empty((D, 3072), dtype=qk_hi.dtype)
    for mt in range(12):
        p = 256 * MT_POS[mt]
        qk_il[:, p:p + 128] = qk_hi[:, 128 * mt:128 * mt + 128]
        qk_il[:, p + 128:p + 256] = qk_lo[:, 128 * mt:128 * mt + 128]
    w8qk = np.ascontiguousarray(
        qk_il.reshape(KT, 128, 3072).transpose(1, 0, 2))   # [128, KT, 3072]
    w8v = np.ascontiguousarray(
        np.concatenate([v_hi, v_lo], axis=1)
        .reshape(KT, 128, 1536).transpose(1, 0, 2))        # [128, KT, 1536]

    wp = np.ascontiguousarray(
        np.asarray(proj_w, np.float32).T.reshape(KT, 128, D)
        .transpose(1, 0, 2)).astype(BF16)

    qb = np.zeros((128, 12), np.float32)
    qb_scaled = (np.asarray(q_bias, np.float32) * (SCALE * SQ)).reshape(KT, 128)
    qb[:, :KT] = qb_scaled.T

    rb = np.asarray(rpb_table, np.float32)[
        np.asarray(rel_index).reshape(-1)].reshape(N, N, NH)   # [q, k, h]
    M = np.exp(rb).transpose(2, 1, 0)                          # [h, k, q]
    Mp = np.zeros((NH, TT * 128, N), np.float32)
    Mp[:, :N] = M
    arr = Mp.reshape(NH, TT, 128, N).transpose(0, 2, 1, 3).astype(BF16)
    bstr = np.ascontiguousarray(arr[NRES:])
    return w8qk, w8v, wp, qb, bstr


def _make_in_maps(inputs):
    x = np.asarray(inputs["x"], dtype=np.float32)
    w8qk, w8v, wp, qb, bstr = _prep_shared(
        inputs["qkv_w"], inputs["q_bias"], inputs["rpb_table"],
        inputs["proj_w"], inputs["rel_index"])

    in_maps = []
    for i in range(NCORES):
        xsl = x[i * BL:(i + 1) * BL]                    # [BL, N, D]
        xT = np.zeros((BL, D, NP), np.float32)
        xT[:, :, :N] = xsl.transpose(0, 2, 1)
        x8 = xT.astype(F8)
        xr8 = (xT - x8.astype(np.float32)).astype(F8)
        x8 = np.ascontiguousarray(
            x8.reshape(BL, KT, 128, NP).transpose(0, 2, 1, 3))
        xr8 = np.ascontiguousarray(
            xr8.reshape(BL, KT, 128, NP).transpose(0, 2, 1, 3))
        in_maps.append({
            "x8": x8, "xr8": xr8, "w8qk": w8qk, "w8v": w8v, "wp": wp,
            "qkvb": qb, "bstr": bstr,
        })
    return in_maps


def _finish(inputs, res):
    out = np.concatenate(
        [np.asarray(res.results[i]["out"], np.float32) for i in range(NCORES)],
        axis=0)
    row = (np.asarray(inputs["v_bias"], np.float32)
           @ np.asarray(inputs["proj_w"], np.float32).T
           + np.asarray(inputs["proj_b"], np.float32))
    out += row[None, None, :]
    return np.ascontiguousarray(out)


def kernel(**inputs):
    in_maps = _make_in_maps(inputs)
    nc = _get_nc()
    res = run_bass_kernel_spmd(nc, in_maps, core_ids=list(range(NCORES)))
    return _finish(inputs, res)


def kernel_traced(**inputs):
    """Like kernel() but also returns (out, BassKernelResults with profile)."""
    in_maps = _make_in_maps(inputs)
    nc = _get_nc()
    res = run_bass_kernel_spmd(nc, in_maps, core_ids=list(range(NCORES)),
                               trace=True)
    return _finish(inputs, res), res


# revision 65
# speedup vs baseline: 1.3884x; 1.0021x over previous
"""BEiT-style windowed attention (B=32, N=577, D=768, 12 heads) on 8 TRN2 cores.

Strategy: pure data-parallel over batch (4 batch elements per core, no
collectives). qkv projection in fp8e4 DoubleRow matmuls with 3-term residual
compensation (hi@hi + lo@hi + hi@lo) for bf16-level accuracy at 1.33x bf16
speed; scores / P@V / out-proj in bf16 (fp8 there fails the 2e-2 gate:
logit-error rms transfers ~1:1 into the output metric). Softmax is unsafe-exp
with the scale constants folded into the fp8 weight encodings
(q: SQ*softmax_scale, k: SK, v: SV) and unfolded via the exp() scale immediate
and the final evacuation.

Per-core dataflow per batch element:
  qkT [d,tok]  = 9 fp8-DR matmuls per 128-col block (x8/xr8 vs W8hi/W8lo)
  v   [tok,d]  = same, transposed roles; kept at SV*v (ones col appended)
  S.T [k,q]    = kT(T) @ qT in bf16 (K=64)
  praw         = exp(S.T * 1/(SQ*SK))  on scalar engine
  P            = praw * exp(rel_bias).T   (DVE / GpSimd split)
  O_un [q,65]  = P(T) @ v_aug  (col 64 = rowsum)
  O            = O_un[:, :64] * recip(rowsum)   (= SV * O_norm)
  OT           = XBAR DMA transpose (SBUF->SBUF)
  out  [tok,d] = OT(T) @ W_projT * 1/SV  -> bf16 -> HBM

v-bias and proj-bias are folded into a host-side row add (out += v_bias @
proj_w.T + proj_b); q-bias is applied in the qk evacuation (per-partition).

Issue order is software-pipelined: batch b+1's qkv matmul groups are
interleaved into batch b's attention heads so the PE never idles waiting on
exp/mul, and the second half of the bias table is streamed from DRAM (first
half resident) to fit SBUF.
"""

import numpy as np
import ml_dtypes

import concourse.bass as bass
import concourse.tile as tile
from concourse import bacc
from concourse import mybir
from concourse.bass_utils import run_bass_kernel_spmd
from concourse.masks import make_identity

B, N, D = 32, 577, 768
NH, DH = 12, 64
NCORES = 8
BL = B // NCORES            # 4 batch elements per core
SCALE = DH ** -0.5
KT = D // 128                # 6 contraction tiles over D
TT = (N + 127) // 128        # 5 token tiles (4x128 + 65)
NP = 592                     # fp8 x token pad (DoubleRow slice stride % 16)
SQ, SK, SV = 256.0, 32.0, 64.0   # fp8 scale folds for q / k / v
NRES = 0                     # bias-table heads resident in SBUF; rest streamed

BF16 = ml_dtypes.bfloat16
F8 = ml_dtypes.float8_e4m3   # TRN e4m3 (max normal 240)

F32 = mybir.dt.float32
BF = mybir.dt.bfloat16
E4 = mybir.dt.float8e4
DR = mybir.MatmulPerfMode.DoubleRow
ADD = mybir.AluOpType.add
MULT = mybir.AluOpType.mult


# weight-column block order = first-use order during the pipelined emission
MT_ORDER = [0, 6, 1, 7, 2, 8, 3, 9, 4, 10, 5, 11]
MT_POS = {m: i for i, m in enumerate(MT_ORDER)}


def tok_m(t):
    return min(128, N - 128 * t)


def _build_nc():
    nc = bacc.Bacc()

    x8_d = nc.declare_dram_parameter("x8", [BL, 128, KT, NP], E4, isOutput=False)
    xr8_d = nc.declare_dram_parameter("xr8", [BL, 128, KT, NP], E4, isOutput=False)
    w8qk_d = nc.declare_dram_parameter("w8qk", [128, KT, 3072], E4, isOutput=False)
    w8v_d = nc.declare_dram_parameter("w8v", [128, KT, 1536], E4, isOutput=False)
    wp_d = nc.declare_dram_parameter("wp", [128, KT, D], BF, isOutput=False)
    qkvb_d = nc.declare_dram_parameter("qkvb", [128, 12], F32, isOutput=False)
    bstr_d = nc.declare_dram_parameter("bstr", [NH - NRES, 128, TT, N], BF,
                                       isOutput=False)
    out_d = nc.declare_dram_parameter("out", [BL, N, D], BF, isOutput=True)

    Exp = mybir.ActivationFunctionType.Exp
    QCH_DR = [(0, 256), (256, 256), (512, N - 512)]   # qk DR out chunks
    VCH_DR = [(0, 256), (256, 256), (512, 256)]       # v DR out chunks
    SCH = [(0, 512), (512, N - 512)]                  # scores bf16 chunks
    DCH = [(0, 512), (512, 256)]                      # proj bf16 chunks

    with tile.TileContext(nc) as tc:
        with (
            tc.tile_pool(name="singles", bufs=1) as singles,
            tc.tile_pool(name="xp", bufs=2) as x_pool,
            tc.tile_pool(name="qktp", bufs=2) as qkt_pool,
            tc.tile_pool(name="vp", bufs=2) as v_pool,
            tc.tile_pool(name="bstrp", bufs=4) as bstr_pool,
            tc.tile_pool(name="prawp", bufs=4) as praw_pool,
            tc.tile_pool(name="expsp", bufs=15) as exps_pool,
            tc.tile_pool(name="op", bufs=2) as o_pool,
            tc.tile_pool(name="otp", bufs=2) as ot_pool,
            tc.tile_pool(name="outp", bufs=2) as out_pool,
            tc.tile_pool(name="smallp", bufs=4) as small_pool,
            tc.tile_pool(name="psA", bufs=3, space="PSUM") as psA,
            tc.tile_pool(name="psB", bufs=2, space="PSUM") as psB,
        ):
            # ---- one-time loads (ordered so batch-0 qk work starts ASAP:
            # x first, then qk weight cols; v cols / bias / proj stream in
            # behind while the first qk matmul groups run) ----
            w8qk = singles.tile([128, KT, 3072], E4)
            w8v = singles.tile([128, KT, 1536], E4)
            qkvb = singles.tile([128, 12], F32)
            wproj = singles.tile([128, KT, D], BF)
            ident = singles.tile([128, 128], BF)

            xs = {}      # b -> (x8 tile, xr8 tile)
            qkts = {}    # b -> qkT tile [128, 12, N] bf16
            vs = {}      # b -> v_sb tile (strided view with ones col)
            os_ = {}     # b -> o_sb tile
            bias_tiles = {}   # (b, h) -> streamed bias tile

            def load_x(b, split=False):
                x8 = x_pool.tile([128, KT, NP], E4, name="x8t", tag="x8")
                xr8 = x_pool.tile([128, KT, NP], E4, name="xr8t", tag="xr8")
                nc.sync.dma_start(out=x8, in_=x8_d[b])
                if not split:
                    nc.sync.dma_start(out=xr8, in_=xr8_d[b])
                xs[b] = (x8, xr8)
                return xr8

            def prefetch_bias(b, h):
                t = bstr_pool.tile([128, TT, N], BF, name="biash")
                nc.sync.dma_start(out=t, in_=bstr_d[h - NRES])
                bias_tiles[(b, h)] = t

            def bias_ap(b, h):
                return bias_tiles[(b, h)]

            def emit_qk_group(b, mt):
                """One 128-col block of the q/k projection: 27 fp8-DR matmuls."""
                x8, xr8 = xs[b]
                if b not in qkts:
                    qkts[b] = qkt_pool.tile([128, 2 * KT, N], BF, name="qkT")
                ps = psA.tile([128, N], F32, name="ps_qk", tag="G", bufs=1)
                cb = 256 * MT_POS[mt]
                for c0, w in QCH_DR:
                    idx = 0
                    for coff, xt in ((0, x8), (128, x8), (0, xr8)):
                        for kp in range(3):
                            nc.tensor.matmul(
                                ps[:, c0:c0 + w],
                                w8qk[:, 2 * kp:2 * kp + 2,
                                     coff + cb:coff + cb + 128],
                                xt[:, 2 * kp:2 * kp + 2, c0:c0 + w],
                                start=(idx == 0), stop=(idx == 8),
                                perf_mode=DR,
                            )
                            idx += 1
                nc.vector.tensor_scalar(
                    out=qkts[b][:, mt, :], in0=ps[:, :N],
                    scalar1=qkvb[:, mt:mt + 1], scalar2=None, op0=ADD,
                )

            def emit_v_group(b, tt):
                """One 128-token block of the v projection (kept at SV*v)."""
                x8, xr8 = xs[b]
                if b not in vs:
                    v_sb = v_pool.tile([128, TT, NH * 65], BF, name="v_sb")
                    v_str = v_sb.rearrange("p t (h c) -> p t h c", c=65)
                    nc.vector.memset(v_str[:, :, :, 64:65], 1.0)
                    vs[b] = v_str
                v_str = vs[b]
                m = tok_m(tt)
                ps = psA.tile([128, D], F32, name="ps_v", tag="G", bufs=1)
                for c0, w in VCH_DR:
                    idx = 0
                    for xt, coff in ((x8, 0), (xr8, 0), (x8, 768)):
                        for kp in range(3):
                            nc.tensor.matmul(
                                ps[:m, c0:c0 + w],
                                xt[:, 2 * kp:2 * kp + 2, 128 * tt:128 * tt + m],
                                w8v[:, 2 * kp:2 * kp + 2, coff + c0:coff + c0 + w],
                                start=(idx == 0), stop=(idx == 8),
                                perf_mode=DR,
                            )
                            idx += 1
                nc.vector.tensor_scalar(
                    out=v_str[:m, tt, :, 0:64],
                    in0=ps[:m, :].rearrange("p (h c) -> p h c", c=64),
                    scalar1=1.0, scalar2=None, op0=MULT,
                )

            def emit_scores(b, h, fill=None):
                """S.T tiles -> exp -> bias-mul for one head; returns expS tiles.

                `fill(n)` emits up to n deferred PE work items; called between
                score tiles so the PE has queued work while psA slots recycle
                at exp() speed.
                """
                qkT = qkts[b]
                po = 64 * (h % 2)
                qT = qkT[po:po + 64, h // 2, :]
                kTh = qkT[po:po + 64, KT + h // 2, :]
                bh = bias_ap(b, h)
                expS = [exps_pool.tile([128, N], BF, name="expS", tag="es")
                        for _ in range(TT)]
                for kt in range(TT):
                    if fill is not None and kt >= 2:
                        # window where scores wait on exp() draining a psA
                        # slot; only psB-based work (P@V of the previous head,
                        # transposes) can actually run here
                        fill(2 if kt > 2 else 1)
                    km = tok_m(kt)
                    ps_s = psA.tile([128, N], F32, name="ps_s", tag="A", bufs=2)
                    for c0, w in SCH:
                        nc.tensor.matmul(
                            ps_s[:km, c0:c0 + w],
                            kTh[:, 128 * kt:128 * kt + km],
                            qT[:, c0:c0 + w],
                            start=True, stop=True,
                        )
                    praw = praw_pool.tile([128, N], BF)
                    nc.scalar.activation(praw[:km, :], ps_s[:km, :], Exp,
                                         scale=1.0 / (SQ * SK))
                    eng = nc.gpsimd
                    eng.tensor_mul(expS[kt][:km, :], praw[:km, :], bh[:km, kt, :])
                return expS

            def emit_pv_piece(b, h, expS, qt):
                if b not in os_:
                    os_[b] = o_pool.tile([128, TT, D], BF, name="o_sb")
                o_sb = os_[b]
                v_str = vs[b]
                # kt order starts at TT-1: the wait on the last bias-mul is
                # consolidated at one fillable point instead of stalling each
                # qt chain mid-accumulation
                kt_order = [TT - 1] + list(range(TT - 1))
                qm = tok_m(qt)
                ps_o = psB.tile([128, 128], F32, name="ps_o", tag="B")
                for i, kt in enumerate(kt_order):
                    km = tok_m(kt)
                    nc.tensor.matmul(
                        ps_o[:qm, :65],
                        expS[kt][:km, 128 * qt:128 * qt + qm],
                        v_str[:km, kt, h, :],
                        start=(i == 0), stop=(i == TT - 1),
                    )
                rcp = small_pool.tile([128, 1], F32)
                nc.vector.reciprocal(rcp[:qm], ps_o[:qm, 64:65])
                nc.vector.tensor_mul(
                    o_sb[:qm, qt, 64 * h:64 * h + 64],
                    ps_o[:qm, 0:64],
                    rcp[:qm, 0:1].to_broadcast([qm, 64]),
                )

            def emit_transpose(b, qt):
                # XBAR DMA transpose: o_sb[q, 768] -> oT[d, kt, q-cols].
                # For qt4 the full 128 partitions are sent; garbage rows land
                # in oT cols >=577 which proj never reads.
                nc.sync.dma_start_transpose(
                    out=ots[b][:, :, 128 * qt:128 * (qt + 1)],
                    in_=os_[b][:, qt, :],
                )

            def emit_proj(b, tt):
                oT = ots[b]
                m = tok_m(tt)
                ps = psA.tile([128, D], F32, name="ps_p", tag="G", bufs=1)
                out_sb = out_pool.tile([128, D], BF)
                # chunk-major so the first chunk's evacuation overlaps the
                # second chunk's matmuls
                for c0, w in DCH:
                    for kt in range(KT):
                        nc.tensor.matmul(
                            ps[:m, c0:c0 + w],
                            oT[:, kt, 128 * tt:128 * tt + m],
                            wproj[:, kt, c0:c0 + w],
                            start=(kt == 0), stop=(kt == KT - 1),
                        )
                    nc.vector.tensor_scalar(
                        out=out_sb[:m, c0:c0 + w], in0=ps[:m, c0:c0 + w],
                        scalar1=1.0 / SV, scalar2=None, op0=MULT,
                    )
                nc.sync.dma_start(out=out_d[b, 128 * tt:128 * tt + m, :],
                                  in_=out_sb[:m, :])

            # ---- software-pipelined emission ----
            # attn(b) is interleaved with qkv(b+1) and transpose+proj(b-1) so
            # the PE stream has fill work while exp/mul latencies drain.
            xr8_0 = load_x(0, split=True)
            nc.sync.dma_start(out=w8qk[:, :, 0:1024], in_=w8qk_d[:, :, 0:1024])
            nc.sync.dma_start(out=xr8_0, in_=xr8_d[0])
            nc.sync.dma_start(out=w8qk[:, :, 1024:2048],
                              in_=w8qk_d[:, :, 1024:2048])
            nc.sync.dma_start(out=qkvb, in_=qkvb_d[:])
            nc.sync.dma_start(out=w8qk[:, :, 2048:3072],
                              in_=w8qk_d[:, :, 2048:3072])
            nc.sync.dma_start(out=w8v, in_=w8v_d[:])
            nc.sync.dma_start(out=wproj, in_=wp_d[:])
            make_identity(nc, ident)

            ots = {}

            def tp_items(b, tail=False):
                ots[b] = ot_pool.tile([128, KT, TT * 128], BF, name="oT",
                                      uniquify=True)
                items = []
                for qt in range(TT):
                    emit_transpose(b, qt)
                items += [(emit_proj, (b, qt)) for qt in range(TT)]
                return items

            # qk-groups for mts needed only from head 4 on are deferred into
            # the OWNING batch's early attention heads, so every attention
            # phase (including the last batch's) has PE fill work.
            LATE_MTS = [2, KT + 2, 3, KT + 3, 4, KT + 4, 5, KT + 5]
            EARLY_MTS = [0, 1, KT, KT + 1]
            deferred = {}   # b -> list of (emit fn, args) pinned to heads 0..3

            for mt in EARLY_MTS:
                emit_qk_group(0, mt)
            deferred[0] = [(emit_qk_group, (0, mt)) for mt in LATE_MTS]

            from collections import deque
            psb_q = deque()   # psB-only work, consumed inside scores windows

            def psb_fill(n):
                for _ in range(n):
                    if psb_q:
                        f, a = psb_q.popleft()
                        f(*a)

            for b in range(BL):
                nxt = []
                if b >= 1:
                    # pv(b-1, h11) pieces are still queued: they fill h0's
                    # windows, then the transpose DMAs launch right after
                    ots[b - 1] = ot_pool.tile([128, KT, TT * 128], BF,
                                              name="oT", uniquify=True)
                    nxt += [(emit_proj, (b - 1, qt)) for qt in range(TT)]
                if b + 1 < BL:
                    load_x(b + 1)
                    nxt += [(emit_qk_group, (b + 1, mt)) for mt in EARLY_MTS]
                    nxt += [(emit_v_group, (b + 1, tt)) for tt in range(TT)]
                    deferred[b + 1] = [(emit_qk_group, (b + 1, mt))
                                       for mt in LATE_MTS]
                for hh in range(NRES, min(NRES + 4, NH)):
                    prefetch_bias(b, hh)
                state = {"ni": 0, "quota": 0}
                mine = deferred.pop(b, [])

                def fill(n):
                    take = min(n, state["quota"], len(nxt) - state["ni"])
                    for _ in range(take):
                        f, a = nxt[state["ni"]]
                        f(*a)
                        state["ni"] += 1
                        state["quota"] -= 1

                for h in range(NH):
                    if NRES + 4 <= h + 4 < NH:
                        prefetch_bias(b, h + 4)
                    state["quota"] = (len(nxt) * (h + 1)) // NH - state["ni"]
                    expS = emit_scores(b, h, psb_fill)
                    if b >= 1 and h == 0:
                        while psb_q:          # any pv(b-1,h11) leftovers
                            f, a = psb_q.popleft()
                            f(*a)
                        for qt in range(TT):
                            emit_transpose(b - 1, qt)
                    if h < len(mine):
                        f, a = mine[h]
                        f(*a)
                    if b == 0 and h == 0:
                        # v(0) deadline is h1's windows; by now w8v has landed
                        for tt in range(TT):
                            emit_v_group(0, tt)
                    fill(len(nxt))
                    # this head's P@V runs inside the NEXT head's scores
                    # windows (it only needs psB slots there)
                    for qt in range(TT):
                        psb_q.append((emit_pv_piece, (b, h, expS, qt)))
                    if b == BL - 1 and h == NH - 1:
                        while psb_q:
                            f, a = psb_q.popleft()
                            f(*a)
                state["quota"] = len(nxt)
                fill(len(nxt))
            for f, a in tp_items(BL - 1, tail=True):
                f(*a)
    nc.finalize()
    return nc


_NC_CACHE = {}


def _get_nc():
    if "nc" not in _NC_CACHE:
        _NC_CACHE["nc"] = _build_nc()
    return _NC_CACHE["nc"]


def _q8(a):
    return np.asarray(a, np.float32).astype(F8)


def _prep_shared(qkv_w, q_bias, rpb_table, proj_w, rel_index):
    qkv_w = np.asarray(qkv_w, dtype=np.float32)
    Wq = qkv_w[:D].T * (SCALE * SQ)          # [768, 768] in-dim major
    Wk = qkv_w[D:2 * D].T * SK
    Wv = qkv_w[2 * D:].T * SV
    qk = np.concatenate([Wq, Wk], axis=1)     # [768, 1536]
    qk_hi = _q8(qk)
    qk_lo = _q8(qk - qk_hi.astype(np.float32))
    v_hi = _q8(Wv)
    v_lo = _q8(Wv - v_hi.astype(np.float32))
    # per-mt interleave [hi_mt | lo_mt] so a small leading DMA covers the
    # first matmul groups
    qk_il = np.empty((D, 3072), dtype=qk_hi.dtype)
    for mt in range(12):
        p = 256 * MT_POS[mt]
        qk_il[:, p:p + 128] = qk_hi[:, 128 * mt:128 * mt + 128]
        qk_il[:, p + 128:p + 256] = qk_lo[:, 128 * mt:128 * mt + 128]
    w8qk = np.ascontiguousarray(
        qk_il.reshape(KT, 128, 3072).transpose(1, 0, 2))   # [128, KT, 3072]
    w8v = np.ascontiguousarray(
        np.concatenate([v_hi, v_lo], axis=1)
        .reshape(KT, 128, 1536).transpose(1, 0, 2))        # [128, KT, 1536]

    wp = np.ascontiguousarray(
        np.asarray(proj_w, np.float32).T.reshape(KT, 128, D)
        .transpose(1, 0, 2)).astype(BF16)

    qb = np.zeros((128, 12), np.float32)
    qb_scaled = (np.asarray(q_bias, np.float32) * (SCALE * SQ)).reshape(KT, 128)
    qb[:, :KT] = qb_scaled.T

    rb = np.asarray(rpb_table, np.float32)[
        np.asarray(rel_index).reshape(-1)].reshape(N, N, NH)   # [q, k, h]
    M = np.exp(rb).transpose(2, 1, 0)                          # [h, k, q]
    Mp = np.zeros((NH, TT * 128, N), np.float32)
    Mp[:, :N] = M
    arr = Mp.reshape(NH, TT, 128, N).transpose(0, 2, 1, 3).astype(BF16)
    bstr = np.ascontiguousarray(arr[NRES:])
    return w8qk, w8v, wp, qb, bstr


def _make_in_maps(inputs):
    x = np.asarray(inputs["x"], dtype=np.float32)
    w8qk, w8v, wp, qb, bstr = _prep_shared(
        inputs["qkv_w"], inputs["q_bias"], inputs["rpb_table"],
        inputs["proj_w"], inputs["rel_index"])

    in_maps = []
    for i in range(NCORES):
        xsl = x[i * BL:(i + 1) * BL]                    # [BL, N, D]
        xT = np.zeros((BL, D, NP), np.float32)
        xT[:, :, :N] = xsl.transpose(0, 2, 1)
        x8 = xT.astype(F8)
        xr8 = (xT - x8.astype(np.float32)).astype(F8)
        x8 = np.ascontiguousarray(
            x8.reshape(BL, KT, 128, NP).transpose(0, 2, 1, 3))
        xr8 = np.ascontiguousarray(
            xr8.reshape(BL, KT, 128, NP).transpose(0, 2, 1, 3))
        in_maps.append({
            "x8": x8, "xr8": xr8, "w8qk": w8qk, "w8v": w8v, "wp": wp,
            "qkvb": qb, "bstr": bstr,
        })
    return in_maps


def _finish(inputs, res):
    out = np.concatenate(
        [np.asarray(res.results[i]["out"], np.float32) for i in range(NCORES)],
        axis=0)
    row = (np.asarray(inputs["v_bias"], np.float32)
           @ np.asarray(inputs["proj_w"], np.float32).T
           + np.asarray(inputs["proj_b"], np.float32))
    out += row[None, None, :]
    return np.ascontiguousarray(out)


def kernel(**inputs):
    in_maps = _make_in_maps(inputs)
    nc = _get_nc()
    res = run_bass_kernel_spmd(nc, in_maps, core_ids=list(range(NCORES)))
    return _finish(inputs, res)


def kernel_traced(**inputs):
    """Like kernel() but also returns (out, BassKernelResults with profile)."""
    in_maps = _make_in_maps(inputs)
    nc = _get_nc()
    res = run_bass_kernel_spmd(nc, in_maps, core_ids=list(range(NCORES)),
                               trace=True)
    return _finish(inputs, res), res


# revision 70
# speedup vs baseline: 1.3902x; 1.0013x over previous
"""BEiT-style windowed attention (B=32, N=577, D=768, 12 heads) on 8 TRN2 cores.

Strategy: pure data-parallel over batch (4 batch elements per core, no
collectives). qkv projection in fp8e4 DoubleRow matmuls with 3-term residual
compensation (hi@hi + lo@hi + hi@lo) for bf16-level accuracy at 1.33x bf16
speed; scores / P@V / out-proj in bf16 (fp8 there fails the 2e-2 gate:
logit-error rms transfers ~1:1 into the output metric). Softmax is unsafe-exp
with the scale constants folded into the fp8 weight encodings
(q: SQ*softmax_scale, k: SK, v: SV) and unfolded via the exp() scale immediate
and the final evacuation.

Per-core dataflow per batch element:
  qkT [d,tok]  = 9 fp8-DR matmuls per 128-col block (x8/xr8 vs W8hi/W8lo)
  v   [tok,d]  = same, transposed roles; kept at SV*v (ones col appended)
  S.T [k,q]    = kT(T) @ qT in bf16 (K=64)
  praw         = exp(S.T * 1/(SQ*SK))  on scalar engine
  P            = praw * exp(rel_bias).T   (DVE / GpSimd split)
  O_un [q,65]  = P(T) @ v_aug  (col 64 = rowsum)
  O            = O_un[:, :64] * recip(rowsum)   (= SV * O_norm)
  OT           = XBAR DMA transpose (SBUF->SBUF)
  out  [tok,d] = OT(T) @ W_projT * 1/SV  -> bf16 -> HBM

v-bias and proj-bias are folded into a host-side row add (out += v_bias @
proj_w.T + proj_b); q-bias is applied in the qk evacuation (per-partition).

Issue order is software-pipelined: batch b+1's qkv matmul groups are
interleaved into batch b's attention heads so the PE never idles waiting on
exp/mul, and the second half of the bias table is streamed from DRAM (first
half resident) to fit SBUF.
"""

import numpy as np
import ml_dtypes

import concourse.bass as bass
import concourse.tile as tile
from concourse import bacc
from concourse import mybir
from concourse.bass_utils import run_bass_kernel_spmd
from concourse.masks import make_identity

B, N, D = 32, 577, 768
NH, DH = 12, 64
NCORES = 8
BL = B // NCORES            # 4 batch elements per core
SCALE = DH ** -0.5
KT = D // 128                # 6 contraction tiles over D
TT = (N + 127) // 128        # 5 token tiles (4x128 + 65)
NP = 592                     # fp8 x token pad (DoubleRow slice stride % 16)
SQ, SK, SV = 256.0, 32.0, 64.0   # fp8 scale folds for q / k / v
NRES = 0                     # bias-table heads resident in SBUF; rest streamed

BF16 = ml_dtypes.bfloat16
F8 = ml_dtypes.float8_e4m3   # TRN e4m3 (max normal 240)

F32 = mybir.dt.float32
BF = mybir.dt.bfloat16
E4 = mybir.dt.float8e4
DR = mybir.MatmulPerfMode.DoubleRow
ADD = mybir.AluOpType.add
MULT = mybir.AluOpType.mult


# weight-column block order = first-use order during the pipelined emission
MT_ORDER = [0, 6, 1, 7, 2, 8, 3, 9, 4, 10, 5, 11]
MT_POS = {m: i for i, m in enumerate(MT_ORDER)}


def tok_m(t):
    return min(128, N - 128 * t)


def _build_nc():
    nc = bacc.Bacc()

    x8_d = nc.declare_dram_parameter("x8", [BL, 128, KT, NP], E4, isOutput=False)
    xr8_d = nc.declare_dram_parameter("xr8", [BL, 128, KT, NP], E4, isOutput=False)
    w8qk_d = nc.declare_dram_parameter("w8qk", [128, KT, 3072], E4, isOutput=False)
    w8v_d = nc.declare_dram_parameter("w8v", [128, KT, 1536], E4, isOutput=False)
    wp_d = nc.declare_dram_parameter("wp", [128, KT, D], BF, isOutput=False)
    qkvb_d = nc.declare_dram_parameter("qkvb", [128, 12], F32, isOutput=False)
    bstr_d = nc.declare_dram_parameter("bstr", [NH - NRES, 128, TT, N], BF,
                                       isOutput=False)
    out_d = nc.declare_dram_parameter("out", [BL, N, D], BF, isOutput=True)

    Exp = mybir.ActivationFunctionType.Exp
    QCH_DR = [(0, 256), (256, 256), (512, N - 512)]   # qk DR out chunks
    VCH_DR = [(0, 256), (256, 256), (512, 256)]       # v DR out chunks
    SCH = [(0, 512), (512, N - 512)]                  # scores bf16 chunks
    DCH = [(0, 512), (512, 256)]                      # proj bf16 chunks

    with tile.TileContext(nc) as tc:
        with (
            tc.tile_pool(name="singles", bufs=1) as singles,
            tc.tile_pool(name="xp", bufs=2) as x_pool,
            tc.tile_pool(name="qktp", bufs=2) as qkt_pool,
            tc.tile_pool(name="vp", bufs=2) as v_pool,
            tc.tile_pool(name="bstrp", bufs=4) as bstr_pool,
            tc.tile_pool(name="prawp", bufs=5) as praw_pool,
            tc.tile_pool(name="expsp", bufs=16) as exps_pool,
            tc.tile_pool(name="op", bufs=2) as o_pool,
            tc.tile_pool(name="otp", bufs=2) as ot_pool,
            tc.tile_pool(name="outp", bufs=2) as out_pool,
            tc.tile_pool(name="smallp", bufs=4) as small_pool,
            tc.tile_pool(name="psA", bufs=3, space="PSUM") as psA,
            tc.tile_pool(name="psB", bufs=2, space="PSUM") as psB,
        ):
            # ---- one-time loads (ordered so batch-0 qk work starts ASAP:
            # x first, then qk weight cols; v cols / bias / proj stream in
            # behind while the first qk matmul groups run) ----
            w8qk = singles.tile([128, KT, 3072], E4)
            w8v = singles.tile([128, KT, 1536], E4)
            qkvb = singles.tile([128, 12], F32)
            wproj = singles.tile([128, KT, D], BF)
            ident = singles.tile([128, 128], BF)

            xs = {}      # b -> (x8 tile, xr8 tile)
            qkts = {}    # b -> qkT tile [128, 12, N] bf16
            vs = {}      # b -> v_sb tile (strided view with ones col)
            os_ = {}     # b -> o_sb tile
            bias_tiles = {}   # (b, h) -> streamed bias tile

            def load_x(b, split=False):
                x8 = x_pool.tile([128, KT, NP], E4, name="x8t", tag="x8")
                xr8 = x_pool.tile([128, KT, NP], E4, name="xr8t", tag="xr8")
                nc.sync.dma_start(out=x8, in_=x8_d[b])
                if not split:
                    nc.sync.dma_start(out=xr8, in_=xr8_d[b])
                xs[b] = (x8, xr8)
                return xr8

            def prefetch_bias(b, h):
                t = bstr_pool.tile([128, TT, N], BF, name="biash")
                nc.sync.dma_start(out=t, in_=bstr_d[h - NRES])
                bias_tiles[(b, h)] = t

            def bias_ap(b, h):
                return bias_tiles[(b, h)]

            def emit_qk_group(b, mt):
                """One 128-col block of the q/k projection: 27 fp8-DR matmuls."""
                x8, xr8 = xs[b]
                if b not in qkts:
                    qkts[b] = qkt_pool.tile([128, 2 * KT, N], BF, name="qkT")
                ps = psA.tile([128, N], F32, name="ps_qk", tag="G", bufs=1)
                cb = 256 * MT_POS[mt]
                for c0, w in QCH_DR:
                    idx = 0
                    for coff, xt in ((0, x8), (128, x8), (0, xr8)):
                        for kp in range(3):
                            nc.tensor.matmul(
                                ps[:, c0:c0 + w],
                                w8qk[:, 2 * kp:2 * kp + 2,
                                     coff + cb:coff + cb + 128],
                                xt[:, 2 * kp:2 * kp + 2, c0:c0 + w],
                                start=(idx == 0), stop=(idx == 8),
                                perf_mode=DR,
                            )
                            idx += 1
                nc.vector.tensor_scalar(
                    out=qkts[b][:, mt, :], in0=ps[:, :N],
                    scalar1=qkvb[:, mt:mt + 1], scalar2=None, op0=ADD,
                )

            def emit_v_group(b, tt):
                """One 128-token block of the v projection (kept at SV*v)."""
                x8, xr8 = xs[b]
                if b not in vs:
                    v_sb = v_pool.tile([128, TT, NH * 65], BF, name="v_sb")
                    v_str = v_sb.rearrange("p t (h c) -> p t h c", c=65)
                    nc.vector.memset(v_str[:, :, :, 64:65], 1.0)
                    vs[b] = v_str
                v_str = vs[b]
                m = tok_m(tt)
                ps = psA.tile([128, D], F32, name="ps_v", tag="G", bufs=1)
                for c0, w in VCH_DR:
                    idx = 0
                    for xt, coff in ((x8, 0), (xr8, 0), (x8, 768)):
                        for kp in range(3):
                            nc.tensor.matmul(
                                ps[:m, c0:c0 + w],
                                xt[:, 2 * kp:2 * kp + 2, 128 * tt:128 * tt + m],
                                w8v[:, 2 * kp:2 * kp + 2, coff + c0:coff + c0 + w],
                                start=(idx == 0), stop=(idx == 8),
                                perf_mode=DR,
                            )
                            idx += 1
                nc.vector.tensor_scalar(
                    out=v_str[:m, tt, :, 0:64],
                    in0=ps[:m, :].rearrange("p (h c) -> p h c", c=64),
                    scalar1=1.0, scalar2=None, op0=MULT,
                )

            def emit_scores(b, h, fill=None):
                """S.T tiles -> exp -> bias-mul for one head; returns expS tiles.

                `fill(n)` emits up to n deferred PE work items; called between
                score tiles so the PE has queued work while psA slots recycle
                at exp() speed.
                """
                qkT = qkts[b]
                po = 64 * (h % 2)
                qT = qkT[po:po + 64, h // 2, :]
                kTh = qkT[po:po + 64, KT + h // 2, :]
                bh = bias_ap(b, h)
                expS = [exps_pool.tile([128, N], BF, name="expS", tag="es")
                        for _ in range(TT)]
                for kt in range(TT):
                    if fill is not None and kt >= 2:
                        # window where scores wait on exp() draining a psA
                        # slot; only psB-based work (P@V of the previous head,
                        # transposes) can actually run here
                        fill(2 if kt > 2 else 1)
                    km = tok_m(kt)
                    ps_s = psA.tile([128, N], F32, name="ps_s", tag="A", bufs=2)
                    for c0, w in SCH:
                        nc.tensor.matmul(
                            ps_s[:km, c0:c0 + w],
                            kTh[:, 128 * kt:128 * kt + km],
                            qT[:, c0:c0 + w],
                            start=True, stop=True,
                        )
                    praw = praw_pool.tile([128, N], BF)
                    nc.scalar.activation(praw[:km, :], ps_s[:km, :], Exp,
                                         scale=1.0 / (SQ * SK))
                    eng = nc.gpsimd
                    eng.tensor_mul(expS[kt][:km, :], praw[:km, :], bh[:km, kt, :])
                return expS

            def emit_pv_piece(b, h, expS, qt):
                if b not in os_:
                    os_[b] = o_pool.tile([128, TT, D], BF, name="o_sb")
                o_sb = os_[b]
                v_str = vs[b]
                # kt order starts at TT-1: the wait on the last bias-mul is
                # consolidated at one fillable point instead of stalling each
                # qt chain mid-accumulation
                kt_order = [TT - 1] + list(range(TT - 1))
                qm = tok_m(qt)
                ps_o = psB.tile([128, 128], F32, name="ps_o", tag="B")
                for i, kt in enumerate(kt_order):
                    km = tok_m(kt)
                    nc.tensor.matmul(
                        ps_o[:qm, :65],
                        expS[kt][:km, 128 * qt:128 * qt + qm],
                        v_str[:km, kt, h, :],
                        start=(i == 0), stop=(i == TT - 1),
                    )
                rcp = small_pool.tile([128, 1], F32)
                nc.vector.reciprocal(rcp[:qm], ps_o[:qm, 64:65])
                nc.vector.tensor_mul(
                    o_sb[:qm, qt, 64 * h:64 * h + 64],
                    ps_o[:qm, 0:64],
                    rcp[:qm, 0:1].to_broadcast([qm, 64]),
                )

            def emit_transpose(b, qt):
                # XBAR DMA transpose: o_sb[q, 768] -> oT[d, kt, q-cols].
                # For qt4 the full 128 partitions are sent; garbage rows land
                # in oT cols >=577 which proj never reads.
                nc.sync.dma_start_transpose(
                    out=ots[b][:, :, 128 * qt:128 * (qt + 1)],
                    in_=os_[b][:, qt, :],
                )

            def emit_proj(b, tt):
                oT = ots[b]
                m = tok_m(tt)
                ps = psA.tile([128, D], F32, name="ps_p", tag="G", bufs=1)
                out_sb = out_pool.tile([128, D], BF)
                # chunk-major so the first chunk's evacuation overlaps the
                # second chunk's matmuls
                for c0, w in DCH:
                    for kt in range(KT):
                        nc.tensor.matmul(
                            ps[:m, c0:c0 + w],
                            oT[:, kt, 128 * tt:128 * tt + m],
                            wproj[:, kt, c0:c0 + w],
                            start=(kt == 0), stop=(kt == KT - 1),
                        )
                    nc.vector.tensor_scalar(
                        out=out_sb[:m, c0:c0 + w], in0=ps[:m, c0:c0 + w],
                        scalar1=1.0 / SV, scalar2=None, op0=MULT,
                    )
                nc.sync.dma_start(out=out_d[b, 128 * tt:128 * tt + m, :],
                                  in_=out_sb[:m, :])

            # ---- software-pipelined emission ----
            # attn(b) is interleaved with qkv(b+1) and transpose+proj(b-1) so
            # the PE stream has fill work while exp/mul latencies drain.
            xr8_0 = load_x(0, split=True)
            nc.sync.dma_start(out=w8qk[:, :, 0:1024], in_=w8qk_d[:, :, 0:1024])
            nc.sync.dma_start(out=xr8_0, in_=xr8_d[0])
            nc.sync.dma_start(out=w8qk[:, :, 1024:2048],
                              in_=w8qk_d[:, :, 1024:2048])
            nc.sync.dma_start(out=qkvb, in_=qkvb_d[:])
            nc.sync.dma_start(out=w8qk[:, :, 2048:3072],
                              in_=w8qk_d[:, :, 2048:3072])
            nc.sync.dma_start(out=w8v, in_=w8v_d[:])
            nc.sync.dma_start(out=wproj, in_=wp_d[:])
            make_identity(nc, ident)

            ots = {}

            def tp_items(b, tail=False):
                ots[b] = ot_pool.tile([128, KT, TT * 128], BF, name="oT",
                                      uniquify=True)
                items = []
                for qt in range(TT):
                    emit_transpose(b, qt)
                items += [(emit_proj, (b, qt)) for qt in range(TT)]
                return items

            # qk-groups for mts needed only from head 4 on are deferred into
            # the OWNING batch's early attention heads, so every attention
            # phase (including the last batch's) has PE fill work.
            LATE_MTS = [2, KT + 2, 3, KT + 3, 4, KT + 4, 5, KT + 5]
            EARLY_MTS = [0, 1, KT, KT + 1]
            deferred = {}   # b -> list of (emit fn, args) pinned to heads 0..3

            for mt in EARLY_MTS:
                emit_qk_group(0, mt)
            deferred[0] = [(emit_qk_group, (0, mt)) for mt in LATE_MTS]

            from collections import deque
            psb_q = deque()   # psB-only work, consumed inside scores windows

            def psb_fill(n):
                for _ in range(n):
                    if psb_q:
                        f, a = psb_q.popleft()
                        f(*a)

            for b in range(BL):
                nxt = []
                if b >= 1:
                    # pv(b-1, h11) pieces are still queued: they fill h0's
                    # windows, then the transpose DMAs launch right after
                    ots[b - 1] = ot_pool.tile([128, KT, TT * 128], BF,
                                              name="oT", uniquify=True)
                    nxt += [(emit_proj, (b - 1, qt)) for qt in range(TT)]
                if b + 1 < BL:
                    load_x(b + 1)
                    nxt += [(emit_qk_group, (b + 1, mt)) for mt in EARLY_MTS]
                    nxt += [(emit_v_group, (b + 1, tt)) for tt in range(TT)]
                    deferred[b + 1] = [(emit_qk_group, (b + 1, mt))
                                       for mt in LATE_MTS]
                for hh in range(NRES, min(NRES + 4, NH)):
                    prefetch_bias(b, hh)
                state = {"ni": 0, "quota": 0}
                mine = deferred.pop(b, [])

                def fill(n):
                    take = min(n, state["quota"], len(nxt) - state["ni"])
                    for _ in range(take):
                        f, a = nxt[state["ni"]]
                        f(*a)
                        state["ni"] += 1
                        state["quota"] -= 1

                for h in range(NH):
                    if NRES + 4 <= h + 4 < NH:
                        prefetch_bias(b, h + 4)
                    state["quota"] = (len(nxt) * (h + 1)) // NH - state["ni"]
                    expS = emit_scores(b, h, psb_fill)
                    if b >= 1 and h == 0:
                        while psb_q:          # any pv(b-1,h11) leftovers
                            f, a = psb_q.popleft()
                            f(*a)
                        for qt in range(TT):
                            emit_transpose(b - 1, qt)
                    if h < len(mine):
                        f, a = mine[h]
                        f(*a)
                    if b == 0 and h == 0:
                        # v(0) deadline is h1's windows; by now w8v has landed
                        for tt in range(TT):
                            emit_v_group(0, tt)
                    fill(len(nxt))
                    # this head's P@V runs inside the NEXT head's scores
                    # windows (it only needs psB slots there)
                    for qt in range(TT):
                        psb_q.append((emit_pv_piece, (b, h, expS, qt)))
                    if b == BL - 1 and h == NH - 1:
                        while psb_q:
                            f, a = psb_q.popleft()
                            f(*a)
                state["quota"] = len(nxt)
                fill(len(nxt))
            for f, a in tp_items(BL - 1, tail=True):
                f(*a)
    nc.finalize()
    return nc


_NC_CACHE = {}


def _get_nc():
    if "nc" not in _NC_CACHE:
        _NC_CACHE["nc"] = _build_nc()
    return _NC_CACHE["nc"]


def _q8(a):
    return np.asarray(a, np.float32).astype(F8)


def _prep_shared(qkv_w, q_bias, rpb_table, proj_w, rel_index):
    qkv_w = np.asarray(qkv_w, dtype=np.float32)
    Wq = qkv_w[:D].T * (SCALE * SQ)          # [768, 768] in-dim major
    Wk = qkv_w[D:2 * D].T * SK
    Wv = qkv_w[2 * D:].T * SV
    qk = np.concatenate([Wq, Wk], axis=1)     # [768, 1536]
    qk_hi = _q8(qk)
    qk_lo = _q8(qk - qk_hi.astype(np.float32))
    v_hi = _q8(Wv)
    v_lo = _q8(Wv - v_hi.astype(np.float32))
    # per-mt interleave [hi_mt | lo_mt] so a small leading DMA covers the
    # first matmul groups
    qk_il = np.empty((D, 3072), dtype=qk_hi.dtype)
    for mt in range(12):
        p = 256 * MT_POS[mt]
        qk_il[:, p:p + 128] = qk_hi[:, 128 * mt:128 * mt + 128]
        qk_il[:, p + 128:p + 256] = qk_lo[:, 128 * mt:128 * mt + 128]
    w8qk = np.ascontiguousarray(
        qk_il.reshape(KT, 128, 3072).transpose(1, 0, 2))   # [128, KT, 3072]
    w8v = np.ascontiguousarray(
        np.concatenate([v_hi, v_lo], axis=1)
        .reshape(KT, 128, 1536).transpose(1, 0, 2))        # [128, KT, 1536]

    wp = np.ascontiguousarray(
        np.asarray(proj_w, np.float32).T.reshape(KT, 128, D)
        .transpose(1, 0, 2)).astype(BF16)

    qb = np.zeros((128, 12), np.float32)
    qb_scaled = (np.asarray(q_bias, np.float32) * (SCALE * SQ)).reshape(KT, 128)
    qb[:, :KT] = qb_scaled.T

    rb = np.asarray(rpb_table, np.float32)[
        np.asarray(rel_index).reshape(-1)].reshape(N, N, NH)   # [q, k, h]
    M = np.exp(rb).transpose(2, 1, 0)                          # [h, k, q]
    Mp = np.zeros((NH, TT * 128, N), np.float32)
    Mp[:, :N] = M
    arr = Mp.reshape(NH, TT, 128, N).transpose(0, 2, 1, 3).astype(BF16)
    bstr = np.ascontiguousarray(arr[NRES:])
    return w8qk, w8v, wp, qb, bstr


def _make_in_maps(inputs):
    x = np.asarray(inputs["x"], dtype=np.float32)
    w8qk, w8v, wp, qb, bstr = _prep_shared(
        inputs["qkv_w"], inputs["q_bias"], inputs["rpb_table"],
        inputs["proj_w"], inputs["rel_index"])

    in_maps = []
    for i in range(NCORES):
        xsl = x[i * BL:(i + 1) * BL]                    # [BL, N, D]
        xT = np.zeros((BL, D, NP), np.float32)
        xT[:, :, :N] = xsl.transpose(0, 2, 1)
        x8 = xT.astype(F8)
        xr8 = (xT - x8.astype(np.float32)).astype(F8)
        x8 = np.ascontiguousarray(
            x8.reshape(BL, KT, 128, NP).transpose(0, 2, 1, 3))
        xr8 = np.ascontiguousarray(
            xr8.reshape(BL, KT, 128, NP).transpose(0, 2, 1, 3))
        in_maps.append({
            "x8": x8, "xr8": xr8, "w8qk": w8qk, "w8v": w8v, "wp": wp,
            "qkvb": qb, "bstr": bstr,
        })
    return in_maps


def _finish(inputs, res):
    out = np.concatenate(
        [np.asarray(res.results[i]["out"], np.float32) for i in range(NCORES)],
        axis=0)
    row = (np.asarray(inputs["v_bias"], np.float32)
           @ np.asarray(inputs["proj_w"], np.float32).T
           + np.asarray(inputs["proj_b"], np.float32))
    out += row[None, None, :]
    return np.ascontiguousarray(out)


def kernel(**inputs):
    in_maps = _make_in_maps(inputs)
    nc = _get_nc()
    res = run_bass_kernel_spmd(nc, in_maps, core_ids=list(range(NCORES)))
    return _finish(inputs, res)


def kernel_traced(**inputs):
    """Like kernel() but also returns (out, BassKernelResults with profile)."""
    in_maps = _make_in_maps(inputs)
    nc = _get_nc()
    res = run_bass_kernel_spmd(nc, in_maps, core_ids=list(range(NCORES)),
                               trace=True)
    return _finish(inputs, res), res


# revision 75
# speedup vs baseline: 1.3963x; 1.0044x over previous
"""BEiT-style windowed attention (B=32, N=577, D=768, 12 heads) on 8 TRN2 cores.

Strategy: pure data-parallel over batch (4 batch elements per core, no
collectives). qkv projection in fp8e4 DoubleRow matmuls with 3-term residual
compensation (hi@hi + lo@hi + hi@lo) for bf16-level accuracy at 1.33x bf16
speed; scores / P@V / out-proj in bf16 (fp8 there fails the 2e-2 gate:
logit-error rms transfers ~1:1 into the output metric). Softmax is unsafe-exp
with the scale constants folded into the fp8 weight encodings
(q: SQ*softmax_scale, k: SK, v: SV) and unfolded via the exp() scale immediate
and the final evacuation.

Per-core dataflow per batch element:
  qkT [d,tok]  = 9 fp8-DR matmuls per 128-col block (x8/xr8 vs W8hi/W8lo)
  v   [tok,d]  = same, transposed roles; kept at SV*v (ones col appended)
  S.T [k,q]    = kT(T) @ qT in bf16 (K=64)
  praw         = exp(S.T * 1/(SQ*SK))  on scalar engine
  P            = praw * exp(rel_bias).T   (DVE / GpSimd split)
  O_un [q,65]  = P(T) @ v_aug  (col 64 = rowsum)
  O            = O_un[:, :64] * recip(rowsum)   (= SV * O_norm)
  OT           = XBAR DMA transpose (SBUF->SBUF)
  out  [tok,d] = OT(T) @ W_projT * 1/SV  -> bf16 -> HBM

v-bias and proj-bias are folded into a host-side row add (out += v_bias @
proj_w.T + proj_b); q-bias is applied in the qk evacuation (per-partition).

Issue order is software-pipelined: batch b+1's qkv matmul groups are
interleaved into batch b's attention heads so the PE never idles waiting on
exp/mul, and the second half of the bias table is streamed from DRAM (first
half resident) to fit SBUF.
"""

import numpy as np
import ml_dtypes

import concourse.bass as bass
import concourse.tile as tile
from concourse import bacc
from concourse import mybir
from concourse.bass_utils import run_bass_kernel_spmd
from concourse.masks import make_identity

B, N, D = 32, 577, 768
NH, DH = 12, 64
NCORES = 8
BL = B // NCORES            # 4 batch elements per core
SCALE = DH ** -0.5
KT = D // 128                # 6 contraction tiles over D
TT = (N + 127) // 128        # 5 token tiles (4x128 + 65)
NP = 592                     # fp8 x token pad (DoubleRow slice stride % 16)
SQ, SK, SV = 256.0, 32.0, 64.0   # fp8 scale folds for q / k / v
NRES = 0                     # bias-table heads resident in SBUF; rest streamed

BF16 = ml_dtypes.bfloat16
F8 = ml_dtypes.float8_e4m3   # TRN e4m3 (max normal 240)

F32 = mybir.dt.float32
BF = mybir.dt.bfloat16
E4 = mybir.dt.float8e4
DR = mybir.MatmulPerfMode.DoubleRow
ADD = mybir.AluOpType.add
MULT = mybir.AluOpType.mult


# weight-column block order = first-use order during the pipelined emission
MT_ORDER = [0, 6, 1, 7, 2, 8, 3, 9, 4, 10, 5, 11]
MT_POS = {m: i for i, m in enumerate(MT_ORDER)}


def tok_m(t):
    return min(128, N - 128 * t)


def _build_nc():
    nc = bacc.Bacc()

    x8_d = nc.declare_dram_parameter("x8", [BL, 128, KT, NP], E4, isOutput=False)
    xr8_d = nc.declare_dram_parameter("xr8", [BL, 128, KT, NP], E4, isOutput=False)
    w8qk_d = nc.declare_dram_parameter("w8qk", [128, KT, 3072], E4, isOutput=False)
    w8v_d = nc.declare_dram_parameter("w8v", [128, KT, 1536], E4, isOutput=False)
    wp_d = nc.declare_dram_parameter("wp", [128, KT, D], BF, isOutput=False)
    qkvb_d = nc.declare_dram_parameter("qkvb", [128, 12], F32, isOutput=False)
    bstr_d = nc.declare_dram_parameter("bstr", [NH - NRES, 128, TT, N], BF,
                                       isOutput=False)
    out_d = nc.declare_dram_parameter("out", [BL, N, D], BF, isOutput=True)

    Exp = mybir.ActivationFunctionType.Exp
    QCH_DR = [(0, 256), (256, 256), (512, N - 512)]   # qk DR out chunks
    VCH_DR = [(0, 256), (256, 256), (512, 256)]       # v DR out chunks
    SCH = [(0, 512), (512, N - 512)]                  # scores bf16 chunks
    DCH = [(0, 512), (512, 256)]                      # proj bf16 chunks

    with tile.TileContext(nc) as tc:
        with (
            tc.tile_pool(name="singles", bufs=1) as singles,
            tc.tile_pool(name="xp", bufs=2) as x_pool,
            tc.tile_pool(name="qktp", bufs=2) as qkt_pool,
            tc.tile_pool(name="vp", bufs=2) as v_pool,
            tc.tile_pool(name="bstrp", bufs=4) as bstr_pool,
            tc.tile_pool(name="prawp", bufs=5) as praw_pool,
            tc.tile_pool(name="expsp", bufs=16) as exps_pool,
            tc.tile_pool(name="op", bufs=2) as o_pool,
            tc.tile_pool(name="otp", bufs=2) as ot_pool,
            tc.tile_pool(name="outp", bufs=2) as out_pool,
            tc.tile_pool(name="smallp", bufs=4) as small_pool,
            tc.tile_pool(name="psA", bufs=3, space="PSUM") as psA,
        ):
            # ---- one-time loads (ordered so batch-0 qk work starts ASAP:
            # x first, then qk weight cols; v cols / bias / proj stream in
            # behind while the first qk matmul groups run) ----
            w8qk = singles.tile([128, KT, 3072], E4)
            w8v = singles.tile([128, KT, 1536], E4)
            qkvb = singles.tile([128, 12], F32)
            wproj = singles.tile([128, KT, D], BF)
            ident = singles.tile([128, 128], BF)

            xs = {}      # b -> (x8 tile, xr8 tile)
            qkts = {}    # b -> qkT tile [128, 12, N] bf16
            vs = {}      # b -> v_sb tile (strided view with ones col)
            os_ = {}     # b -> o_sb tile
            bias_tiles = {}   # (b, h) -> streamed bias tile

            def load_x(b, split=False):
                x8 = x_pool.tile([128, KT, NP], E4, name="x8t", tag="x8")
                xr8 = x_pool.tile([128, KT, NP], E4, name="xr8t", tag="xr8")
                nc.sync.dma_start(out=x8, in_=x8_d[b])
                if not split:
                    nc.sync.dma_start(out=xr8, in_=xr8_d[b])
                xs[b] = (x8, xr8)
                return xr8

            def prefetch_bias(b, h):
                t = bstr_pool.tile([128, TT, N], BF, name="biash")
                nc.sync.dma_start(out=t, in_=bstr_d[h - NRES])
                bias_tiles[(b, h)] = t

            def bias_ap(b, h):
                return bias_tiles[(b, h)]

            def emit_qk_group(b, mt):
                """One 128-col block of the q/k projection: 27 fp8-DR matmuls."""
                x8, xr8 = xs[b]
                if b not in qkts:
                    qkts[b] = qkt_pool.tile([128, 2 * KT, N], BF, name="qkT")
                cb = 256 * MT_POS[mt]
                ps1 = psA.tile([128, 512], F32, name="ps_qk1", tag="G", bufs=2)
                ps2 = psA.tile([128, 128], F32, name="ps_qk2", tag="G", bufs=2)
                for c0, w in QCH_DR:
                    ps, p0 = (ps1, c0) if c0 < 512 else (ps2, 0)
                    idx = 0
                    for coff, xt in ((0, x8), (128, x8), (0, xr8)):
                        for kp in range(3):
                            nc.tensor.matmul(
                                ps[:, p0:p0 + w],
                                w8qk[:, 2 * kp:2 * kp + 2,
                                     coff + cb:coff + cb + 128],
                                xt[:, 2 * kp:2 * kp + 2, c0:c0 + w],
                                start=(idx == 0), stop=(idx == 8),
                                perf_mode=DR,
                            )
                            idx += 1
                nc.vector.tensor_scalar(
                    out=qkts[b][:, mt, 0:512], in0=ps1[:, :],
                    scalar1=qkvb[:, mt:mt + 1], scalar2=None, op0=ADD,
                )
                nc.vector.tensor_scalar(
                    out=qkts[b][:, mt, 512:N], in0=ps2[:, :N - 512],
                    scalar1=qkvb[:, mt:mt + 1], scalar2=None, op0=ADD,
                )

            def emit_v_group(b, tt):
                """One 128-token block of the v projection (kept at SV*v)."""
                x8, xr8 = xs[b]
                if b not in vs:
                    v_sb = v_pool.tile([128, TT, NH * 65], BF, name="v_sb")
                    v_str = v_sb.rearrange("p t (h c) -> p t h c", c=65)
                    nc.vector.memset(v_str[:, :, :, 64:65], 1.0)
                    vs[b] = v_str
                v_str = vs[b]
                m = tok_m(tt)
                ps1 = psA.tile([128, 512], F32, name="ps_v1", tag="G", bufs=2)
                ps2 = psA.tile([128, 256], F32, name="ps_v2", tag="G", bufs=2)
                for c0, w in VCH_DR:
                    ps, p0 = (ps1, c0) if c0 < 512 else (ps2, 0)
                    idx = 0
                    for xt, coff in ((x8, 0), (xr8, 0), (x8, 768)):
                        for kp in range(3):
                            nc.tensor.matmul(
                                ps[:m, p0:p0 + w],
                                xt[:, 2 * kp:2 * kp + 2, 128 * tt:128 * tt + m],
                                w8v[:, 2 * kp:2 * kp + 2, coff + c0:coff + c0 + w],
                                start=(idx == 0), stop=(idx == 8),
                                perf_mode=DR,
                            )
                            idx += 1
                nc.vector.tensor_scalar(
                    out=v_str[:m, tt, 0:8, 0:64],
                    in0=ps1[:m, :].rearrange("p (h c) -> p h c", c=64),
                    scalar1=1.0, scalar2=None, op0=MULT,
                )
                nc.vector.tensor_scalar(
                    out=v_str[:m, tt, 8:12, 0:64],
                    in0=ps2[:m, :].rearrange("p (h c) -> p h c", c=64),
                    scalar1=1.0, scalar2=None, op0=MULT,
                )

            def emit_scores(b, h, fill=None):
                """S.T tiles -> exp -> bias-mul for one head; returns expS tiles.

                `fill(n)` emits up to n deferred PE work items; called between
                score tiles so the PE has queued work while psA slots recycle
                at exp() speed.
                """
                qkT = qkts[b]
                po = 64 * (h % 2)
                qT = qkT[po:po + 64, h // 2, :]
                kTh = qkT[po:po + 64, KT + h // 2, :]
                bh = bias_ap(b, h)
                expS = [exps_pool.tile([128, N], BF, name="expS", tag="es")
                        for _ in range(TT)]
                for kt in range(TT):
                    if fill is not None and kt >= 2:
                        # window where scores wait on exp() draining a psA
                        # slot; only psB-based work (P@V of the previous head,
                        # transposes) can actually run here
                        fill(2 if kt > 2 else 1)
                    km = tok_m(kt)
                    ps_s = psA.tile([128, N], F32, name="ps_s", tag="A", bufs=3)
                    for c0, w in SCH:
                        nc.tensor.matmul(
                            ps_s[:km, c0:c0 + w],
                            kTh[:, 128 * kt:128 * kt + km],
                            qT[:, c0:c0 + w],
                            start=True, stop=True,
                        )
                    praw = praw_pool.tile([128, N], BF)
                    nc.scalar.activation(praw[:km, :], ps_s[:km, :], Exp,
                                         scale=1.0 / (SQ * SK))
                    eng = nc.gpsimd
                    eng.tensor_mul(expS[kt][:km, :], praw[:km, :], bh[:km, kt, :])
                return expS

            def emit_pv_piece(b, h, expS, qt):
                if b not in os_:
                    os_[b] = o_pool.tile([128, TT, D], BF, name="o_sb")
                o_sb = os_[b]
                v_str = vs[b]
                # kt order starts at TT-1: the wait on the last bias-mul is
                # consolidated at one fillable point instead of stalling each
                # qt chain mid-accumulation
                kt_order = [TT - 1] + list(range(TT - 1))
                qm = tok_m(qt)
                ps_o = psA.tile([128, 128], F32, name="ps_o", tag="G", bufs=2)
                for i, kt in enumerate(kt_order):
                    km = tok_m(kt)
                    nc.tensor.matmul(
                        ps_o[:qm, :65],
                        expS[kt][:km, 128 * qt:128 * qt + qm],
                        v_str[:km, kt, h, :],
                        start=(i == 0), stop=(i == TT - 1),
                    )
                rcp = small_pool.tile([128, 1], F32)
                nc.vector.reciprocal(rcp[:qm], ps_o[:qm, 64:65])
                nc.vector.tensor_mul(
                    o_sb[:qm, qt, 64 * h:64 * h + 64],
                    ps_o[:qm, 0:64],
                    rcp[:qm, 0:1].to_broadcast([qm, 64]),
                )

            def emit_transpose(b, qt):
                # XBAR DMA transpose: o_sb[q, 768] -> oT[d, kt, q-cols].
                # For qt4 the full 128 partitions are sent; garbage rows land
                # in oT cols >=577 which proj never reads.
                nc.sync.dma_start_transpose(
                    out=ots[b][:, :, 128 * qt:128 * (qt + 1)],
                    in_=os_[b][:, qt, :],
                )

            def emit_proj(b, tt):
                oT = ots[b]
                m = tok_m(tt)
                out_sb = out_pool.tile([128, D], BF)
                # chunk-major with per-chunk 1-bank psums: first chunk's
                # evacuation overlaps the second chunk's matmuls
                for c0, w in DCH:
                    ps = psA.tile([128, 512], F32, name="ps_p", tag="G",
                                  bufs=2)
                    for kt in range(KT):
                        nc.tensor.matmul(
                            ps[:m, :w],
                            oT[:, kt, 128 * tt:128 * tt + m],
                            wproj[:, kt, c0:c0 + w],
                            start=(kt == 0), stop=(kt == KT - 1),
                        )
                    nc.vector.tensor_scalar(
                        out=out_sb[:m, c0:c0 + w], in0=ps[:m, :w],
                        scalar1=1.0 / SV, scalar2=None, op0=MULT,
                    )
                nc.sync.dma_start(out=out_d[b, 128 * tt:128 * tt + m, :],
                                  in_=out_sb[:m, :])

            # ---- software-pipelined emission ----
            # attn(b) is interleaved with qkv(b+1) and transpose+proj(b-1) so
            # the PE stream has fill work while exp/mul latencies drain.
            xr8_0 = load_x(0, split=True)
            nc.sync.dma_start(out=w8qk[:, :, 0:1024], in_=w8qk_d[:, :, 0:1024])
            nc.sync.dma_start(out=xr8_0, in_=xr8_d[0])
            nc.sync.dma_start(out=w8qk[:, :, 1024:2048],
                              in_=w8qk_d[:, :, 1024:2048])
            nc.sync.dma_start(out=qkvb, in_=qkvb_d[:])
            nc.sync.dma_start(out=w8qk[:, :, 2048:3072],
                              in_=w8qk_d[:, :, 2048:3072])
            nc.sync.dma_start(out=w8v, in_=w8v_d[:])
            nc.sync.dma_start(out=wproj, in_=wp_d[:])
            make_identity(nc, ident)

            ots = {}

            def tp_items(b, tail=False):
                ots[b] = ot_pool.tile([128, KT, TT * 128], BF, name="oT",
                                      uniquify=True)
                items = []
                for qt in range(TT):
                    emit_transpose(b, qt)
                items += [(emit_proj, (b, qt)) for qt in range(TT)]
                return items

            # qk-groups for mts needed only from head 4 on are deferred into
            # the OWNING batch's early attention heads, so every attention
            # phase (including the last batch's) has PE fill work.
            LATE_MTS = [2, KT + 2, 3, KT + 3, 4, KT + 4, 5, KT + 5]
            EARLY_MTS = [0, 1, KT, KT + 1]
            deferred = {}   # b -> list of (emit fn, args) pinned to heads 0..3

            for mt in EARLY_MTS:
                emit_qk_group(0, mt)
            deferred[0] = [(emit_qk_group, (0, mt)) for mt in LATE_MTS]

            from collections import deque
            psb_q = deque()   # psB-only work, consumed inside scores windows

            def psb_fill(n):
                for _ in range(n):
                    if psb_q:
                        f, a = psb_q.popleft()
                        f(*a)

            for b in range(BL):
                nxt = []
                if b >= 1:
                    # pv(b-1, h11) pieces are still queued: they fill h0's
                    # windows, then the transpose DMAs launch right after
                    ots[b - 1] = ot_pool.tile([128, KT, TT * 128], BF,
                                              name="oT", uniquify=True)
                    nxt += [(emit_proj, (b - 1, qt)) for qt in range(TT)]
                if b + 1 < BL:
                    load_x(b + 1)
                    nxt += [(emit_qk_group, (b + 1, mt)) for mt in EARLY_MTS]
                    nxt += [(emit_v_group, (b + 1, tt)) for tt in range(TT)]
                    deferred[b + 1] = [(emit_qk_group, (b + 1, mt))
                                       for mt in LATE_MTS]
                for hh in range(NRES, min(NRES + 4, NH)):
                    prefetch_bias(b, hh)
                state = {"ni": 0, "quota": 0}
                mine = deferred.pop(b, [])

                def fill(n):
                    take = min(n, state["quota"], len(nxt) - state["ni"])
                    for _ in range(take):
                        f, a = nxt[state["ni"]]
                        f(*a)
                        state["ni"] += 1
                        state["quota"] -= 1

                for h in range(NH):
                    if NRES + 4 <= h + 4 < NH:
                        prefetch_bias(b, h + 4)
                    state["quota"] = (len(nxt) * (h + 1)) // NH - state["ni"]
                    expS = emit_scores(b, h, psb_fill)
                    if b >= 1 and h == 0:
                        while psb_q:          # any pv(b-1,h11) leftovers
                            f, a = psb_q.popleft()
                            f(*a)
                        for qt in range(TT):
                            emit_transpose(b - 1, qt)
                    if h < len(mine):
                        f, a = mine[h]
                        f(*a)
                    if b == 0 and h == 0:
                        # v(0) deadline is h1's windows; by now w8v has landed
                        for tt in range(TT):
                            emit_v_group(0, tt)
                    fill(len(nxt))
                    # this head's P@V runs inside the NEXT head's scores
                    # windows (it only needs psB slots there)
                    for qt in range(TT):
                        psb_q.append((emit_pv_piece, (b, h, expS, qt)))
                    if b == BL - 1 and h == NH - 1:
                        while psb_q:
                            f, a = psb_q.popleft()
                            f(*a)
                state["quota"] = len(nxt)
                fill(len(nxt))
            for f, a in tp_items(BL - 1, tail=True):
                f(*a)
    nc.finalize()
    return nc


_NC_CACHE = {}


def _get_nc():
    if "nc" not in _NC_CACHE:
        _NC_CACHE["nc"] = _build_nc()
    return _NC_CACHE["nc"]


def _q8(a):
    return np.asarray(a, np.float32).astype(F8)


def _prep_shared(qkv_w, q_bias, rpb_table, proj_w, rel_index):
    qkv_w = np.asarray(qkv_w, dtype=np.float32)
    Wq = qkv_w[:D].T * (SCALE * SQ)          # [768, 768] in-dim major
    Wk = qkv_w[D:2 * D].T * SK
    Wv = qkv_w[2 * D:].T * SV
    qk = np.concatenate([Wq, Wk], axis=1)     # [768, 1536]
    qk_hi = _q8(qk)
    qk_lo = _q8(qk - qk_hi.astype(np.float32))
    v_hi = _q8(Wv)
    v_lo = _q8(Wv - v_hi.astype(np.float32))
    # per-mt interleave [hi_mt | lo_mt] so a small leading DMA covers the
    # first matmul groups
    qk_il = np.empty((D, 3072), dtype=qk_hi.dtype)
    for mt in range(12):
        p = 256 * MT_POS[mt]
        qk_il[:, p:p + 128] = qk_hi[:, 128 * mt:128 * mt + 128]
        qk_il[:, p + 128:p + 256] = qk_lo[:, 128 * mt:128 * mt + 128]
    w8qk = np.ascontiguousarray(
        qk_il.reshape(KT, 128, 3072).transpose(1, 0, 2))   # [128, KT, 3072]
    w8v = np.ascontiguousarray(
        np.concatenate([v_hi, v_lo], axis=1)
        .reshape(KT, 128, 1536).transpose(1, 0, 2))        # [128, KT, 1536]

    wp = np.ascontiguousarray(
        np.asarray(proj_w, np.float32).T.reshape(KT, 128, D)
        .transpose(1, 0, 2)).astype(BF16)

    qb = np.zeros((128, 12), np.float32)
    qb_scaled = (np.asarray(q_bias, np.float32) * (SCALE * SQ)).reshape(KT, 128)
    qb[:, :KT] = qb_scaled.T

    rb = np.asarray(rpb_table, np.float32)[
        np.asarray(rel_index).reshape(-1)].reshape(N, N, NH)   # [q, k, h]
    M = np.exp(rb).transpose(2, 1, 0)                          # [h, k, q]
    Mp = np.zeros((NH, TT * 128, N), np.float32)
    Mp[:, :N] = M
    arr = Mp.reshape(NH, TT, 128, N).transpose(0, 2, 1, 3).astype(BF16)
    bstr = np.ascontiguousarray(arr[NRES:])
    return w8qk, w8v, wp, qb, bstr


def _make_in_maps(inputs):
    x = np.asarray(inputs["x"], dtype=np.float32)
    w8qk, w8v, wp, qb, bstr = _prep_shared(
        inputs["qkv_w"], inputs["q_bias"], inputs["rpb_table"],
        inputs["proj_w"], inputs["rel_index"])

    in_maps = []
    for i in range(NCORES):
        xsl = x[i * BL:(i + 1) * BL]                    # [BL, N, D]
        xT = np.zeros((BL, D, NP), np.float32)
        xT[:, :, :N] = xsl.transpose(0, 2, 1)
        x8 = xT.astype(F8)
        xr8 = (xT - x8.astype(np.float32)).astype(F8)
        x8 = np.ascontiguousarray(
            x8.reshape(BL, KT, 128, NP).transpose(0, 2, 1, 3))
        xr8 = np.ascontiguousarray(
            xr8.reshape(BL, KT, 128, NP).transpose(0, 2, 1, 3))
        in_maps.append({
            "x8": x8, "xr8": xr8, "w8qk": w8qk, "w8v": w8v, "wp": wp,
            "qkvb": qb, "bstr": bstr,
        })
    return in_maps


def _finish(inputs, res):
    out = np.concatenate(
        [np.asarray(res.results[i]["out"], np.float32) for i in range(NCORES)],
        axis=0)
    row = (np.asarray(inputs["v_bias"], np.float32)
           @ np.asarray(inputs["proj_w"], np.float32).T
           + np.asarray(inputs["proj_b"], np.float32))
    out += row[None, None, :]
    return np.ascontiguousarray(out)


def kernel(**inputs):
    in_maps = _make_in_maps(inputs)
    nc = _get_nc()
    res = run_bass_kernel_spmd(nc, in_maps, core_ids=list(range(NCORES)))
    return _finish(inputs, res)


def kernel_traced(**inputs):
    """Like kernel() but also returns (out, BassKernelResults with profile)."""
    in_maps = _make_in_maps(inputs)
    nc = _get_nc()
    res = run_bass_kernel_spmd(nc, in_maps, core_ids=list(range(NCORES)),
                               trace=True)
    return _finish(inputs, res), res
